# revision 1
# baseline (speedup 1.0000x reference)
"""Trainium2 Bass kernel for a pre-LN transformer block (B=2,S=2048,H=1024,NH=16,FFN=4096).

Sharding: 8 cores, 512 tokens/core (4 cores per batch element). K/V are
exchanged within each batch group via one 4-rank AllGather (bf16). All matmuls
run in bf16 on the PE array with fp32 PSUM accumulation; LayerNorm statistics,
residuals and the final output stay fp32.

Self-contained: hardcodes shapes; builds the Bass program once and runs it via
run_bass_kernel_spmd on cores 0-7.
"""

import sys

for _p in ("/root/.axon_site/_ro/trn_rl_repo", "/opt/trn_rl_repo"):
    if _p not in sys.path:
        sys.path.append(_p)

import numpy as np
import ml_dtypes

# If BASS_TRACE is set but the axon NTFF hook module is missing, the trace
# path would crash on import; pre-register a no-op hook shim so tracing
# degrades gracefully instead.
try:
    import antenv.axon_hooks  # noqa: F401
except ImportError:
    import types as _types
    _m = _types.ModuleType("antenv.axon_hooks")
    _m._hook = None
    _m.get_axon_ntff_profile_hook = lambda: _m._hook
    _m.set_axon_ntff_profile_hook = lambda h: setattr(_m, "_hook", h)
    sys.modules["antenv.axon_hooks"] = _m

import bass_rust
import concourse.bass as bass
import concourse.mybir as mybir
import concourse.tile as tile
from concourse.bass_utils import run_bass_kernel_spmd

BF16 = mybir.dt.bfloat16
F32 = mybir.dt.float32
AF = mybir.ActivationFunctionType
NPBF16 = np.dtype(ml_dtypes.bfloat16)

B, S, H, NH, DH, FFN = 2, 2048, 1024, 16, 64, 4096
NC = 8                      # cores
T = 512                     # tokens per core
NT = T // 128               # token tiles per core (4)
GROUPS = [[0, 1, 2, 3], [4, 5, 6, 7]]
G = 4                       # cores per batch group
SKEYS = S                   # keys per batch (2048)
NKT = SKEYS // 128          # key tiles (16)
NHP = NH // 2               # head pairs (8)
EPS = 1e-3
VW = DH + 1                 # 65: V columns + ones column per head
KV_CHUNK = T * H            # bf16 elems per (kT | v) contribution: 524288
# key tiles in half-A (first V AllGather) then half-B order
KT_HALF_A = [g * 4 + j for g in range(4) for j in (0, 1)]
KT_HALF_B = [g * 4 + 2 + j for g in range(4) for j in (0, 1)]
KT_PAIRS = [tuple(KT_HALF_A[i:i + 2]) for i in range(0, 8, 2)] + \
           [tuple(KT_HALF_B[i:i + 2]) for i in range(0, 8, 2)]

# ---------------------------------------------------------------------------
# Workaround: this walrus build rejects >1 inline sync-wait per instruction.
# After Tile scheduling, move excess waits onto single-wait NoOp carriers
# inserted immediately before the over-limit instruction (same engine, same
# block, so per-engine program order and wait semantics are preserved).
# ---------------------------------------------------------------------------
def _split_multiwait(nc, limit=1):
    n_new = 0
    for f in nc.m.functions:
        for blk in f.blocks:
            insts = blk.instructions
            out = []
            for ins in insts:
                si = getattr(ins, "sync_info", None)
                waits = list(si.on_wait) if si is not None else []
                if len(waits) > limit:
                    for i, w in enumerate(waits[:-limit]):
                        nop = mybir.InstNoOp(
                            name=f"{ins.name}_w{i}",
                            sync_info=mybir.SyncInfo(on_wait=[w], on_update=[]),
                            bass_nofuse=True,
                            engine=ins.engine,
                        )
                        out.append(nop)
                        n_new += 1
                    ins.sync_info = mybir.SyncInfo(
                        on_wait=waits[-limit:], on_update=list(si.on_update)
                    )
                out.append(ins)
            if len(out) != len(insts):
                blk.instructions = out
    return n_new


def _emit(tc, nc, io):
    """Emit the per-core program. io: dict of DRAM APs."""
    from contextlib import ExitStack

    x_d = io["x"]
    out_d = io["out"]

    # ---- long-lived pools. Tile pools must close in LIFO order: keep all
    # persistent tensors in one outer pool (held to the end); each phase's
    # scratch lives in phase-local pools that close before the next opens. ----
    s_outer = ExitStack()

    constp = s_outer.enter_context(tc.tile_pool(name="constp", bufs=1))
    dramp = s_outer.enter_context(tc.tile_pool(name="dramp", bufs=1, space="DRAM"))

    # constants / biases
    ident = constp.tile([128, 128], BF16)
    nc.sync.dma_start(ident[:], io["ident"][:])
    ones_row = constp.tile([1, 128], BF16)
    nc.sync.dma_start(ones_row[:], io["ones_row"][:])
    bq = constp.tile([128, 8], F32); nc.sync.dma_start(bq[:], io["bq"][:])
    bk = constp.tile([128, 8], F32); nc.sync.dma_start(bk[:], io["bk"][:])
    bi = constp.tile([128, 32], F32); nc.sync.dma_start(bi[:], io["bi"][:])
    bv_bf = constp.tile([1, H], BF16); nc.sync.dma_start(bv_bf[:], io["bv_bf"][:])
    bproj_bf = constp.tile([1, H], BF16); nc.sync.dma_start(bproj_bf[:], io["bproj_bf"][:])
    bo_bf = constp.tile([1, H], BF16); nc.sync.dma_start(bo_bf[:], io["bo_bf"][:])
    eps_t = constp.tile([128, 1], F32); nc.gpsimd.memset(eps_t[:], float(EPS))

    # collective buffers (AllGather concatenates along dim 0); both K and V
    # are gathered in two halves so attention can start on the first half
    cc_k_in_a = dramp.tile([512, T], BF16)
    cc_k_in_b = dramp.tile([512, T], BF16)
    cc_k_out_a = dramp.tile([G * 512, T], BF16)
    cc_k_out_b = dramp.tile([G * 512, T], BF16)
    cc_v_in_a = dramp.tile([256, H], BF16)
    cc_v_in_b = dramp.tile([256, H], BF16)
    cc_v_out_a = dramp.tile([G * 256, H], BF16)
    cc_v_out_b = dramp.tile([G * 256, H], BF16)

    # persistent activations (one outer pool, held until the end)
    persp = s_outer.enter_context(tc.tile_pool(name="persp", bufs=1))
    x2_all = persp.tile([128, NT * H], F32, name="x2_all")
    h3T_all = persp.tile([128, 32 * T], BF16, name="h3T_all")
    h2T_all = persp.tile([128, 8 * T], BF16, name="h2T_all")
    ctxT_all = persp.tile([128, 8 * T], BF16, name="ctxT_all")
    wproj_sb = persp.tile([128, 8 * H], BF16, name="wproj_sb")
    x_all = persp.tile([128, NT * H], F32, name="x_all")
    qT_all = persp.tile([128, 8 * T], BF16, name="qT_all")

    def layer_norm_to(pool, h_out_slice, x_slice):
        """x_slice [128,H] f32 -> h_out_slice [128,H] bf16 standardized."""
        sq = pool.tile([128, H], F32, tag="ln_sq")
        nsum = pool.tile([128, 1], F32, tag="ln_nsum")
        s2 = pool.tile([128, 1], F32, tag="ln_s2")
        var = pool.tile([128, 1], F32, tag="ln_var")
        std = pool.tile([128, 1], F32, tag="ln_std")
        rs = pool.tile([128, 1], F32, tag="ln_rs")
        nmu = pool.tile([128, 1], F32, tag="ln_nmu")
        nmurs = pool.tile([128, 1], F32, tag="ln_nmurs")
        nc.vector.reduce_sum(nsum[:], x_slice, axis=mybir.AxisListType.X, negate=True)
        nc.vector.tensor_mul(sq[:], x_slice, x_slice)
        nc.vector.reduce_sum(s2[:], sq[:], axis=mybir.AxisListType.X)
        nc.vector.tensor_scalar_mul(nmu[:], nsum[:], 1.0 / H)      # -mean
        nc.vector.tensor_scalar_mul(s2[:], s2[:], 1.0 / H)         # E[x^2]
        nc.vector.tensor_mul(var[:], nmu[:], nmu[:])               # mean^2
        nc.vector.tensor_sub(var[:], s2[:], var[:])                # var
        nc.scalar.activation(std[:], var[:], AF.Sqrt, bias=eps_t[:])
        nc.vector.reciprocal(rs[:], std[:])
        nc.vector.tensor_mul(nmurs[:], nmu[:], rs[:])              # -mean*rs
        nc.scalar.activation(h_out_slice, x_slice, AF.Identity, bias=nmurs[:], scale=rs[:])

    def transpose_128(dst_slice, src_slice, tps, cpool):
        """PE-transpose src [128,128] bf16 -> dst [128,128] bf16."""
        ps = tps.tile([128, 128], BF16, tag="tp")
        nc.tensor.transpose(ps[:], src_slice, ident[:])
        nc.vector.tensor_copy(dst_slice, ps[:])

    # =====================================================================
    # Phase A: load x, LN1, h1^T, q^T/k^T (feature-major), v (token-major)
    # =====================================================================
    sA = ExitStack()
    wq_p = sA.enter_context(tc.tile_pool(name="wq_p", bufs=2))
    lnp = sA.enter_context(tc.tile_pool(name="lnp", bufs=2))
    h1p = sA.enter_context(tc.tile_pool(name="h1p", bufs=1))
    h1Tp = sA.enter_context(tc.tile_pool(name="h1Tp", bufs=1))
    tpsA = sA.enter_context(tc.tile_pool(name="tpsA", bufs=2, space="PSUM"))
    mmpsA = sA.enter_context(tc.tile_pool(name="mmpsA", bufs=2, space="PSUM"))
    stgA = sA.enter_context(tc.tile_pool(name="stgA", bufs=4))

    h1_all = h1p.tile([128, NT * H], BF16)
    h1T_all = h1Tp.tile([128, 8 * T], BF16)

    # x + wk load first so the K^T -> AllGather chain is not queued behind
    # the other weight DMAs
    for t in range(NT):
        nc.sync.dma_start(x_all[:, t * H:(t + 1) * H], x_d[t * 128:(t + 1) * 128, :])
    wk_sb = wq_p.tile([128, 8 * H], BF16, tag="w3", name="wk_sb")
    for fb in range(8):
        nc.sync.dma_start(wk_sb[:, fb * H:(fb + 1) * H], io["wk"][fb * 128:(fb + 1) * 128, :])

    for t in range(NT):
        layer_norm_to(lnp, h1_all[:, t * H:(t + 1) * H], x_all[:, t * H:(t + 1) * H])
        for fb in range(8):
            transpose_128(
                h1T_all[:, fb * T + t * 128: fb * T + (t + 1) * 128],
                h1_all[:, t * H + fb * 128: t * H + (fb + 1) * 128],
                tpsA, stgA,
            )

    # k^T feature-major: [128 feats, T] per col-tile; AllGather ASAP
    for ct in range(8):
        ps = mmpsA.tile([128, T], F32, tag="mm_qk")
        for fb in range(8):
            nc.tensor.matmul(
                ps[:],
                wk_sb[:, fb * H + ct * 128: fb * H + (ct + 1) * 128],
                h1T_all[:, fb * T:(fb + 1) * T],
                start=(fb == 0), stop=(fb == 7),
            )
        ktmp = stgA.tile([128, T], BF16, tag="ktmp")
        nc.scalar.activation(ktmp[:], ps[:], AF.Identity, bias=bk[:, ct:ct + 1])
        dst = cc_k_in_a if ct < 4 else cc_k_in_b
        nc.sync.dma_start(dst[(ct % 4) * 128:(ct % 4 + 1) * 128, :], ktmp[:])

    nc.gpsimd.collective_compute(
        "AllGather", mybir.AluOpType.bypass, replica_groups=GROUPS,
        ins=[cc_k_in_a.opt()], outs=[cc_k_out_a.opt()],
    )

    # q^T feature-major (runs while the K AllGather is in flight)
    wq_sb = wq_p.tile([128, 8 * H], BF16, tag="w3", name="wq_sb")
    for fb in range(8):
        nc.sync.dma_start(wq_sb[:, fb * H:(fb + 1) * H], io["wq"][fb * 128:(fb + 1) * 128, :])
    for ct in range(8):
        ps = mmpsA.tile([128, T], F32, tag="mm_qk")
        for fb in range(8):
            nc.tensor.matmul(
                ps[:],
                wq_sb[:, fb * H + ct * 128: fb * H + (ct + 1) * 128],
                h1T_all[:, fb * T:(fb + 1) * T],
                start=(fb == 0), stop=(fb == 7),
            )
        nc.scalar.activation(
            qT_all[:, ct * T:(ct + 1) * T], ps[:], AF.Identity,
            bias=bq[:, ct:ct + 1])

    # v token-major: [128 tok, H] (wv reuses the wk slot once kT is done)
    wv_sb = wq_p.tile([128, 8 * H], BF16, tag="w3", name="wv_sb")
    for fb in range(8):
        nc.sync.dma_start(wv_sb[:, fb * H:(fb + 1) * H], io["wv"][fb * 128:(fb + 1) * 128, :])
    for t in range(NT):
        for cc in range(2):
            ps = mmpsA.tile([128, 512], F32, tag="mm_v")
            for fb in range(8):
                nc.tensor.matmul(
                    ps[:],
                    h1T_all[:, fb * T + t * 128: fb * T + (t + 1) * 128],
                    wv_sb[:, fb * H + cc * 512: fb * H + (cc + 1) * 512],
                    start=(fb == 0), stop=False,
                )
            nc.tensor.matmul(ps[:], ones_row[:], bv_bf[:, cc * 512:(cc + 1) * 512],
                             start=False, stop=True)
            vtmp = stgA.tile([128, 512], BF16, tag="vtmp")
            nc.vector.tensor_copy(vtmp[:], ps[:])
            dst = cc_v_in_a if t < 2 else cc_v_in_b
            nc.sync.dma_start(dst[(t % 2) * 128:(t % 2 + 1) * 128, cc * 512:(cc + 1) * 512], vtmp[:])

    nc.gpsimd.collective_compute(
        "AllGather", mybir.AluOpType.bypass, replica_groups=GROUPS,
        ins=[cc_v_in_a.opt()], outs=[cc_v_out_a.opt()],
    )
    nc.gpsimd.collective_compute(
        "AllGather", mybir.AluOpType.bypass, replica_groups=GROUPS,
        ins=[cc_v_in_b.opt()], outs=[cc_v_out_b.opt()],
    )
    nc.gpsimd.collective_compute(
        "AllGather", mybir.AluOpType.bypass, replica_groups=GROUPS,
        ins=[cc_k_in_b.opt()], outs=[cc_k_out_b.opt()],
    )

    sA.close()

    # prefetch proj weights during attention
    for hp in range(8):
        nc.sync.dma_start(wproj_sb[:, hp * H:(hp + 1) * H],
                          io["wproj"][hp * 128:(hp + 1) * 128, :])

    # =====================================================================
    # Phase B: attention. scores^T per key-tile (row-packed head pairs),
    # exp on ACT, ctx^T via V'=[V|ones] (M=65), normalize with 1/sumexp.
    # =====================================================================
    sB = ExitStack()
    vstg = sB.enter_context(tc.tile_pool(name="vstg", bufs=4))
    kpool = sB.enter_context(tc.tile_pool(name="kpool", bufs=3))
    spool = sB.enter_context(tc.tile_pool(name="spool", bufs=2, space="PSUM"))
    cpool = sB.enter_context(tc.tile_pool(name="cpool", bufs=2, space="PSUM"))
    ppool = sB.enter_context(tc.tile_pool(name="ppool", bufs=8))
    rpool = sB.enter_context(tc.tile_pool(name="rpool", bufs=2))
    vsb = sB.enter_context(tc.tile_pool(name="vsb_p", bufs=1)).tile([128, NKT * NH * VW], BF16, name="vsb")

    def load_kt(hp):
        kt_hp = kpool.tile([128, SKEYS], BF16, tag="kt_hp", name="kt_hp")
        cko, hpo = (cc_k_out_a, hp) if hp < 4 else (cc_k_out_b, hp - 4)
        for g in range(G):
            nc.sync.dma_start(kt_hp[:, g * T:(g + 1) * T],
                              cko[g * 512 + hpo * 128: g * 512 + (hpo + 1) * 128, :])
        return kt_hp

    # ones columns for all key tiles up front (DVE; no data deps)
    for kt in range(NKT):
        blk = vsb[:, kt * NH * VW:(kt + 1) * NH * VW]
        nc.vector.memset(blk.rearrange("p (h x) -> p h x", x=VW)[:, :, DH:VW], 1.0)

    def load_v_half(half_kts, cc_v_out):
        # V from AllGather output, interleaving a ones column per head
        for kt in half_kts:
            g, j = kt // 4, (kt % 4) % 2
            vplain = vstg.tile([128, H], BF16, tag="vplain", name="vplain")
            nc.sync.dma_start(vplain[:], cc_v_out[g * 256 + j * 128: g * 256 + (j + 1) * 128, :])
            blk = vsb[:, kt * NH * VW:(kt + 1) * NH * VW]
            dst = blk.rearrange("p (h x) -> p h x", x=VW)
            nc.vector.tensor_copy(dst[:, :, 0:DH], vplain[:].rearrange("p (h d) -> p h d", d=DH))

    # SP-queue emission order: kt prefetches for the first head pairs, then
    # the V halves (gated on the later AllGathers), then the rest
    kt_tiles = [None] * NHP
    for hp in range(4):
        kt_tiles[hp] = load_kt(hp)
    load_v_half(KT_HALF_A, cc_v_out_a)
    load_v_half(KT_HALF_B, cc_v_out_b)
    for hp in range(4, NHP):
        kt_tiles[hp] = load_kt(hp)

    for hp in range(NHP):
        kt_hp = kt_tiles[hp]
        cps0 = cpool.tile([128, T], F32, tag="ctx0")
        cps1 = cpool.tile([128, T], F32, tag="ctx1")

        def emit_ctx(kt, pb, first, last):
            for h, cps in enumerate((cps0, cps1)):
                head = hp * 2 + h
                nc.tensor.matmul(
                    cps[0:VW, :],
                    vsb[:, kt * NH * VW + head * VW: kt * NH * VW + (head + 1) * VW],
                    pb[:, h * 512:(h + 1) * 512],
                    start=first, stop=last,
                )

        # software-pipelined: scores(kt) | ctx(kt-1) | exp(kt).  Both heads'
        # scores share one [128,1024] psum so a single exp releases the slot
        # and the row-tiled pair stays adjacent (concurrent on the PE).
        KT_ORDER = KT_HALF_A + KT_HALF_B
        prev = None
        for pi, kt in enumerate(KT_ORDER):
            ps = spool.tile([128, 1024], F32, tag="ps", name="ps")
            nc.tensor.matmul(
                ps[:, 0:512],
                kt_hp[0:64, kt * 128:(kt + 1) * 128],
                qT_all[0:64, hp * T:(hp + 1) * T],
                start=True, stop=True, tile_position=(0, 0),
            )
            nc.tensor.matmul(
                ps[:, 512:1024],
                kt_hp[64:128, kt * 128:(kt + 1) * 128],
                qT_all[64:128, hp * T:(hp + 1) * T],
                start=True, stop=True, tile_position=(64, 0),
            )
            if prev is not None:
                emit_ctx(prev[0], prev[1], prev[2], False)
            pb = ppool.tile([128, 1024], BF16, tag="pb", name="pb")
            nc.scalar.activation(pb[:], ps[:], AF.Exp)
            prev = (kt, pb, pi == 0)
        emit_ctx(prev[0], prev[1], prev[2], True)
        rc0 = rpool.tile([1, T], F32, tag="rc0")
        rc1 = rpool.tile([1, T], F32, tag="rc1")
        nc.vector.reciprocal(rc0[:], cps0[DH:VW, :])
        nc.vector.reciprocal(rc1[:], cps1[DH:VW, :])
        rcb0 = rpool.tile([1, T], BF16, tag="rcb0")
        rcb1 = rpool.tile([1, T], BF16, tag="rcb1")
        nc.vector.tensor_copy(rcb0[:], rc0[:])
        nc.vector.tensor_copy(rcb1[:], rc1[:])
        # broadcast 1/sumexp across 64 partitions per head via K=1 matmuls
        bb = spool.tile([128, T], F32, tag="ps", name="bb")
        nc.tensor.matmul(bb[0:64, :], ones_row[:, 0:64], rcb0[:],
                         start=True, stop=True, tile_position=(0, 0))
        nc.tensor.matmul(bb[64:128, :], ones_row[:, 0:64], rcb1[:],
                         start=True, stop=True, tile_position=(0, 64))
        rb = rpool.tile([128, T], F32, tag="rb")
        nc.vector.tensor_copy(rb[:], bb[:])
        nc.vector.tensor_mul(ctxT_all[0:64, hp * T:(hp + 1) * T], cps0[0:DH, :], rb[0:64, :])
        nc.vector.tensor_mul(ctxT_all[64:128, hp * T:(hp + 1) * T], cps1[0:DH, :], rb[64:128, :])

    sB.close()

    # =====================================================================
    # Phase C: proj (token-major) + residual -> x2, LN2 -> h2^T
    # =====================================================================
    sC = ExitStack()
    lnp2 = sC.enter_context(tc.tile_pool(name="lnp2", bufs=2))
    h2p = sC.enter_context(tc.tile_pool(name="h2p", bufs=1))
    tpsC = sC.enter_context(tc.tile_pool(name="tpsC", bufs=2, space="PSUM"))
    mmpsC = sC.enter_context(tc.tile_pool(name="mmpsC", bufs=2, space="PSUM"))
    stgC = sC.enter_context(tc.tile_pool(name="stgC", bufs=4))

    h2_all = h2p.tile([128, NT * H], BF16)

    for t in range(NT):
        for cc in range(2):
            ps = mmpsC.tile([128, 512], F32, tag="pj")
            for hp in range(8):
                nc.tensor.matmul(
                    ps[:],
                    ctxT_all[:, hp * T + t * 128: hp * T + (t + 1) * 128],
                    wproj_sb[:, hp * H + cc * 512: hp * H + (cc + 1) * 512],
                    start=(hp == 0), stop=False,
                )
            nc.tensor.matmul(ps[:], ones_row[:], bproj_bf[:, cc * 512:(cc + 1) * 512],
                             start=False, stop=True)
            nc.vector.tensor_add(
                x2_all[:, t * H + cc * 512: t * H + (cc + 1) * 512],
                ps[:], x_all[:, t * H + cc * 512: t * H + (cc + 1) * 512])
        layer_norm_to(lnp2, h2_all[:, t * H:(t + 1) * H], x2_all[:, t * H:(t + 1) * H])
        for fb in range(8):
            transpose_128(
                h2T_all[:, fb * T + t * 128: fb * T + (t + 1) * 128],
                h2_all[:, t * H + fb * 128: t * H + (fb + 1) * 128],
                tpsC, stgC,
            )

    sC.close()

    # =====================================================================
    # Phase D+E fused: per g-tile: wi matmuls + gelu -> h3T[g], then wo
    # matmuls for output columns 0:512 accumulate into 4 persistent psums.
    # Second pass re-reads h3T for output columns 512:1024.
    # =====================================================================
    sD = ExitStack()
    wip = sD.enter_context(tc.tile_pool(name="wip", bufs=6))
    wop = sD.enter_context(tc.tile_pool(name="wop", bufs=6))
    mmpsD = sD.enter_context(tc.tile_pool(name="mmpsD", bufs=4, space="PSUM"))
    wops = sD.enter_context(tc.tile_pool(name="wops", bufs=1, space="PSUM"))
    outp = sD.enter_context(tc.tile_pool(name="outp", bufs=2))

    NG = FFN // 128  # 32
    psE = [wops.tile([128, 512], F32, tag=f"wo_ps{t}", name=f"wo_ps{t}") for t in range(NT)]
    for g in range(NG):
        wi_g = wip.tile([128, 8, 128], BF16, tag="wi_g", name="wi_g")
        src = io["wi"][g:g + 1, :, :, :].rearrange("o p f c -> (o p) f c")
        nc.sync.dma_start(wi_g[:], src)
        ps = mmpsD.tile([128, T], F32, tag="wi_ps", name="wi_ps")
        for fb in range(8):
            nc.tensor.matmul(
                ps[:], wi_g[:, fb, :], h2T_all[:, fb * T:(fb + 1) * T],
                start=(fb == 0), stop=(fb == 7),
            )
        nc.scalar.activation(h3T_all[:, g * T:(g + 1) * T], ps[:],
                             AF.Gelu_apprx_tanh, bias=bi[:, g:g + 1])
        wo_g = wop.tile([128, 512], BF16, tag="wo_g", name="wo_g")
        nc.sync.dma_start(wo_g[:], io["wo"][g * 128:(g + 1) * 128, 0:512])
        for t in range(NT):
            nc.tensor.matmul(
                psE[t][:],
                h3T_all[:, g * T + t * 128: g * T + (t + 1) * 128],
                wo_g[:],
                start=(g == 0), stop=False,
            )
    for t in range(NT):
        nc.tensor.matmul(psE[t][:], ones_row[:], bo_bf[:, 0:512], start=False, stop=True)
        ot = outp.tile([128, 512], F32, tag="ot", name="ot")
        nc.vector.tensor_add(ot[:], psE[t][:], x2_all[:, t * H: t * H + 512])
        nc.sync.dma_start(out_d[t * 128:(t + 1) * 128, 0:512], ot[:])

    # second pass: output columns 512:1024
    psE2 = [wops.tile([128, 512], F32, tag=f"wo_ps{t}", name=f"wo2_ps{t}") for t in range(NT)]
    for g in range(NG):
        wo_g = wop.tile([128, 512], BF16, tag="wo_g", name="wo_g2")
        nc.sync.dma_start(wo_g[:], io["wo"][g * 128:(g + 1) * 128, 512:1024])
        for t in range(NT):
            nc.tensor.matmul(
                psE2[t][:],
                h3T_all[:, g * T + t * 128: g * T + (t + 1) * 128],
                wo_g[:],
                start=(g == 0), stop=False,
            )
    for t in range(NT):
        nc.tensor.matmul(psE2[t][:], ones_row[:], bo_bf[:, 512:1024], start=False, stop=True)
        ot = outp.tile([128, 512], F32, tag="ot", name="ot2")
        nc.vector.tensor_add(ot[:], psE2[t][:], x2_all[:, t * H + 512: t * H + 1024])
        nc.sync.dma_start(out_d[t * 128:(t + 1) * 128, 512:1024], ot[:])

    sD.close()
    s_outer.close()


def _build_program():
    nc = bass.Bass("TRN2", target_bir_lowering=False, debug=False, num_devices=NC)
    io = {}
    io["x"] = nc.dram_tensor("x", [T, H], F32, kind="ExternalInput").ap()
    io["wq"] = nc.dram_tensor("wq", [H, H], BF16, kind="ExternalInput").ap()
    io["wk"] = nc.dram_tensor("wk", [H, H], BF16, kind="ExternalInput").ap()
    io["wv"] = nc.dram_tensor("wv", [H, H], BF16, kind="ExternalInput").ap()
    io["wproj"] = nc.dram_tensor("wproj", [H, H], BF16, kind="ExternalInput").ap()
    io["wi"] = nc.dram_tensor("wi", [FFN // 128, 128, 8, 128], BF16, kind="ExternalInput").ap()
    io["wo"] = nc.dram_tensor("wo", [FFN, H], BF16, kind="ExternalInput").ap()
    io["bq"] = nc.dram_tensor("bq", [128, 8], F32, kind="ExternalInput").ap()
    io["bk"] = nc.dram_tensor("bk", [128, 8], F32, kind="ExternalInput").ap()
    io["bi"] = nc.dram_tensor("bi", [128, 32], F32, kind="ExternalInput").ap()
    io["bv_bf"] = nc.dram_tensor("bv_bf", [1, H], BF16, kind="ExternalInput").ap()
    io["bproj_bf"] = nc.dram_tensor("bproj_bf", [1, H], BF16, kind="ExternalInput").ap()
    io["bo_bf"] = nc.dram_tensor("bo_bf", [1, H], BF16, kind="ExternalInput").ap()
    io["ident"] = nc.dram_tensor("ident", [128, 128], BF16, kind="ExternalInput").ap()
    io["ones_row"] = nc.dram_tensor("ones_row", [1, 128], BF16, kind="ExternalInput").ap()
    io["out"] = nc.dram_tensor("out", [T, H], F32, kind="ExternalOutput").ap()
    with tile.TileContext(nc) as tc:
        _emit(tc, nc, io)
    _split_multiwait(nc)
    return nc


_PROGRAM = None
LAST_RESULTS = None


def kernel(x, ln1_scale, ln1_bias, qkv_w, qkv_b, proj_w, proj_b,
           ln2_scale, ln2_bias, wi_w, wi_b, wo_w, wo_b):
    global _PROGRAM, LAST_RESULTS
    x = np.asarray(x, np.float32)
    ln1_scale = np.asarray(ln1_scale, np.float32); ln1_bias = np.asarray(ln1_bias, np.float32)
    qkv_w = np.asarray(qkv_w, np.float32); qkv_b = np.asarray(qkv_b, np.float32)
    proj_w = np.asarray(proj_w, np.float32); proj_b = np.asarray(proj_b, np.float32)
    ln2_scale = np.asarray(ln2_scale, np.float32); ln2_bias = np.asarray(ln2_bias, np.float32)
    wi_w = np.asarray(wi_w, np.float32); wi_b = np.asarray(wi_b, np.float32)
    wo_w = np.asarray(wo_w, np.float32); wo_b = np.asarray(wo_b, np.float32)

    # fold LN affine params into the next matmul's weights/biases
    qkv_w_eff = ln1_scale[:, None] * qkv_w
    qkv_b_eff = qkv_b + ln1_bias @ qkv_w
    w3 = qkv_w_eff.reshape(H, NH, 3, DH)
    b3 = qkv_b_eff.reshape(NH, 3, DH)
    scale = 1.0 / np.sqrt(np.float32(DH))
    wq = (w3[:, :, 0, :] * scale).reshape(H, H)
    wk = w3[:, :, 1, :].reshape(H, H)
    wv = w3[:, :, 2, :].reshape(H, H)
    bq_v = (b3[:, 0, :] * scale).reshape(H)
    bk_v = b3[:, 1, :].reshape(H)
    bv_v = b3[:, 2, :].reshape(H)
    wi_eff = ln2_scale[:, None] * wi_w
    bi_v = wi_b + ln2_bias @ wi_w

    common = {
        "wq": wq.astype(NPBF16), "wk": wk.astype(NPBF16), "wv": wv.astype(NPBF16),
        "wproj": proj_w.astype(NPBF16),
        "wi": np.ascontiguousarray(
            wi_eff.astype(NPBF16).reshape(8, 128, 32, 128).transpose(2, 1, 0, 3)),
        "wo": wo_w.astype(NPBF16),
        "bq": np.ascontiguousarray(bq_v.reshape(8, 128).T.astype(np.float32)),
        "bk": np.ascontiguousarray(bk_v.reshape(8, 128).T.astype(np.float32)),
        "bi": np.ascontiguousarray(bi_v.reshape(32, 128).T.astype(np.float32)),
        "bv_bf": bv_v.reshape(1, H).astype(NPBF16),
        "bproj_bf": proj_b.reshape(1, H).astype(NPBF16),
        "bo_bf": wo_b.reshape(1, H).astype(NPBF16),
        "ident": np.eye(128, dtype=NPBF16),
        "ones_row": np.ones((1, 128), NPBF16),
    }
    x_flat = x.reshape(B * S, H)
    in_maps = []
    for c in range(NC):
        m = dict(common)
        m["x"] = np.ascontiguousarray(x_flat[c * T:(c + 1) * T, :])
        in_maps.append(m)

    if _PROGRAM is None:
        _PROGRAM = _build_program()
    r = run_bass_kernel_spmd(_PROGRAM, in_maps, list(range(NC)))
    LAST_RESULTS = r
    out = np.concatenate([r.results[c]["out"] for c in range(NC)], axis=0)
    return out.reshape(B, S, H).astype(np.float32)



# revision 6
# speedup vs baseline: 1.0485x; 1.0485x over previous
"""Trainium2 Bass kernel for a pre-LN transformer block (B=2,S=2048,H=1024,NH=16,FFN=4096).

Sharding: 8 cores, 512 tokens/core (4 cores per batch element). K/V are
exchanged within each batch group via four 4-rank AllGathers (K and V each in
two head-halves) so attention on early head-pairs overlaps the later gathers.
All matmuls run in bf16 on the PE array with fp32 PSUM accumulation; LayerNorm
statistics, residuals and the final output stay fp32.

Self-contained: hardcodes shapes; builds the Bass program once and runs it via
run_bass_kernel_spmd on cores 0-7.
"""

import sys

for _p in ("/root/.axon_site/_ro/trn_rl_repo", "/opt/trn_rl_repo"):
    if _p not in sys.path:
        sys.path.append(_p)

import numpy as np
import ml_dtypes

# If BASS_TRACE is set but the axon NTFF hook module is missing, the trace
# path would crash on import; pre-register a no-op hook shim so tracing
# degrades gracefully instead.
try:
    import antenv.axon_hooks  # noqa: F401
except ImportError:
    import types as _types
    _m = _types.ModuleType("antenv.axon_hooks")
    _m._hook = None
    _m.get_axon_ntff_profile_hook = lambda: _m._hook
    _m.set_axon_ntff_profile_hook = lambda h: setattr(_m, "_hook", h)
    sys.modules["antenv.axon_hooks"] = _m

import bass_rust
import concourse.bass as bass
import concourse.mybir as mybir
import concourse.tile as tile
from concourse.bass_utils import run_bass_kernel_spmd

BF16 = mybir.dt.bfloat16
F32 = mybir.dt.float32
AF = mybir.ActivationFunctionType
NPBF16 = np.dtype(ml_dtypes.bfloat16)

B, S, H, NH, DH, FFN = 2, 2048, 1024, 16, 64, 4096
NC = 8                      # cores
T = 512                     # tokens per core
NT = T // 128               # token tiles per core (4)
GROUPS = [[0, 1, 2, 3], [4, 5, 6, 7]]
G = 4                       # cores per batch group
NKT = 16                    # key tiles per batch (4 ranks x 4)
NHP = NH // 2               # head pairs (8)
EPS = 1e-3
VW = DH + 1                 # 65: V columns + ones column per head
NG = FFN // 128             # 32 ffn row-groups

USE_DMA_TRANSPOSE = True    # xbar DMA transpose for h1T/h2T (fallback: PE)

# ---------------------------------------------------------------------------
# Workaround: this walrus build rejects >1 inline sync-wait per instruction.
# After Tile scheduling, move excess waits onto single-wait NoOp carriers
# inserted immediately before the over-limit instruction (same engine, same
# block, so per-engine program order and wait semantics are preserved).
# ---------------------------------------------------------------------------
def _split_multiwait(nc, limit=1):
    n_new = 0
    for f in nc.m.functions:
        for blk in f.blocks:
            insts = blk.instructions
            out = []
            for ins in insts:
                si = getattr(ins, "sync_info", None)
                waits = list(si.on_wait) if si is not None else []
                if len(waits) > limit:
                    for i, w in enumerate(waits[:-limit]):
                        nop = mybir.InstNoOp(
                            name=f"{ins.name}_w{i}",
                            sync_info=mybir.SyncInfo(on_wait=[w], on_update=[]),
                            bass_nofuse=True,
                            engine=ins.engine,
                        )
                        out.append(nop)
                        n_new += 1
                    ins.sync_info = mybir.SyncInfo(
                        on_wait=waits[-limit:], on_update=list(si.on_update)
                    )
                out.append(ins)
            if len(out) != len(insts):
                blk.instructions = out
    return n_new


def _emit(tc, nc, io):
    """Emit the per-core program. io: dict of DRAM APs."""
    from contextlib import ExitStack

    x_d = io["x"]
    out_d = io["out"]

    s_outer = ExitStack()

    constp = s_outer.enter_context(tc.tile_pool(name="constp", bufs=1))
    dramp = s_outer.enter_context(tc.tile_pool(name="dramp", bufs=1, space="DRAM"))

    # ---- phase-A bulk loads, one descriptor-dense DMA each (SP queue) ----
    persp = s_outer.enter_context(tc.tile_pool(name="persp", bufs=1))
    x_all = persp.tile([128, NT, H], F32, name="x_all")
    nc.sync.dma_start(x_all[:], x_d.rearrange("(t p) h -> p t h", p=128))

    sA = ExitStack()
    wpool = sA.enter_context(tc.tile_pool(name="wpool", bufs=3))
    wk_sb = wpool.tile([128, 8, H], BF16, tag="w3", name="wk_sb")
    nc.sync.dma_start(wk_sb[:], io["wk"].rearrange("(f p) h -> p f h", p=128))

    # constants / biases
    ident = constp.tile([128, 128], BF16)
    nc.sync.dma_start(ident[:], io["ident"][:])
    ones_row = constp.tile([1, 128], BF16)
    nc.sync.dma_start(ones_row[:], io["ones_row"][:])
    # bqki: cols 0:8 = bq, 8:16 = bk, 16:48 = bi (all [128, n])
    bqki = constp.tile([128, 48], F32)
    nc.sync.dma_start(bqki[:], io["bqki"][:])
    # bvpo: [1, 3H] bf16: bv | bproj | bo
    bvpo = constp.tile([1, 3 * H], BF16)
    nc.sync.dma_start(bvpo[:], io["bvpo"][:])
    eps_t = constp.tile([128, 1], F32)
    nc.gpsimd.memset(eps_t[:], float(EPS))

    wq_sb = wpool.tile([128, 8, H], BF16, tag="w3", name="wq_sb")
    nc.sync.dma_start(wq_sb[:], io["wq"].rearrange("(f p) h -> p f h", p=128))
    wv_sb = wpool.tile([128, 8, H], BF16, tag="w3", name="wv_sb")
    nc.sync.dma_start(wv_sb[:], io["wv"].rearrange("(f p) h -> p f h", p=128))

    # persistent activations
    x2_all = persp.tile([128, NT, H], F32, name="x2_all")
    qT_all = persp.tile([128, 8, T], BF16, name="qT_all")
    ctxT_all = persp.tile([128, 8 * T], BF16, name="ctxT_all")
    h2T_all = persp.tile([128, 8, T], BF16, name="h2T_all")
    wproj_sb = persp.tile([128, 8, H], BF16, name="wproj_sb")

    # collective buffers (AllGather concatenates along dim 0)
    cc_k_in_a = dramp.tile([512, T], BF16)     # kT ct0-3 (heads 0-7)
    cc_k_in_b = dramp.tile([512, T], BF16)     # kT ct4-7 (heads 8-15)
    cc_k_out_a = dramp.tile([G * 512, T], BF16)
    cc_k_out_b = dramp.tile([G * 512, T], BF16)
    cc_v_in_a = dramp.tile([T, 512], BF16)     # v cols 0:512 (heads 0-7)
    cc_v_in_b = dramp.tile([T, 512], BF16)     # v cols 512:1024 (heads 8-15)
    cc_v_out_a = dramp.tile([G * T, 512], BF16)
    cc_v_out_b = dramp.tile([G * T, 512], BF16)

    def layer_norm_stats(pool, x_slice):
        """x_slice [128,H] f32 -> (rs [128,1], nmr [128,1]) in SBUF."""
        stats = pool.tile([128, 2, 6], F32, tag="ln_st")
        mv = pool.tile([128, 2], F32, tag="ln_mv")
        std = pool.tile([128, 1], F32, tag="ln_std")
        rs = pool.tile([128, 1], F32, tag="ln_rs")
        nmr = pool.tile([128, 1], F32, tag="ln_nmr")
        xc = x_slice.rearrange("p (n c) -> p n c", c=512)
        nc.vector.bn_stats(out=stats[:, 0, :], in_=xc[:, 0, :])
        nc.vector.bn_stats(out=stats[:, 1, :], in_=xc[:, 1, :])
        nc.vector.bn_aggr(out=mv[:], in_=stats[:])
        nc.scalar.activation(std[:], mv[:, 1:2], AF.Sqrt, bias=eps_t[:])
        nc.vector.reciprocal(rs[:], std[:])
        nc.vector.tensor_mul(nmr[:], mv[:, 0:1], rs[:])
        nc.vector.tensor_scalar_mul(nmr[:], nmr[:], -1.0)
        return rs, nmr

    # =====================================================================
    # Phase A: load x, LN1 -> h1, h1T (xbar transpose), kT, v, qT.
    # K/V AllGathers (by head-half) dispatched as soon as inputs land.
    # =====================================================================
    lnp = sA.enter_context(tc.tile_pool(name="lnp", bufs=2))
    h1p = sA.enter_context(tc.tile_pool(name="h1p", bufs=2))
    h1Tp = sA.enter_context(tc.tile_pool(name="h1Tp", bufs=1))
    ktp = sA.enter_context(tc.tile_pool(name="ktp", bufs=1))
    vlp = sA.enter_context(tc.tile_pool(name="vlp", bufs=2))
    mmpsA = sA.enter_context(tc.tile_pool(name="mmpsA", bufs=3, space="PSUM"))
    tpsA = None
    stgA = None
    if not USE_DMA_TRANSPOSE:
        tpsA = sA.enter_context(tc.tile_pool(name="tpsA", bufs=2, space="PSUM"))
        stgA = sA.enter_context(tc.tile_pool(name="stgA", bufs=2))

    h1T_all = h1Tp.tile([128, 8, T], BF16, name="h1T_all")
    ktA_sb = ktp.tile([128, 4, T], BF16, name="ktA_sb")
    ktB_sb = ktp.tile([128, 4, T], BF16, name="ktB_sb")

    def transpose_tile(dstT_all, h_tile, t):
        """h_tile [128, H] bf16 -> dstT_all[:, fb, t*128:(t+1)*128] for all fb."""
        dst = dstT_all.rearrange("p f (tt c) -> p f tt c", c=128)[:, :, t, :]
        if USE_DMA_TRANSPOSE:
            nc.scalar.dma_start_transpose(dst, h_tile)
        else:
            for fb in range(8):
                ps = tpsA.tile([128, 128], BF16, tag="tp")
                nc.tensor.transpose(ps[:], h_tile[:, fb * 128:(fb + 1) * 128], ident[:])
                nc.vector.tensor_copy(dst[:, fb, :], ps[:])

    for t in range(NT):
        rs, nmr = layer_norm_stats(lnp, x_all[:, t, :])
        h1 = h1p.tile([128, H], BF16, tag="h1")
        nc.scalar.activation(h1[:], x_all[:, t, :], AF.Identity, bias=nmr[:], scale=rs[:])
        transpose_tile(h1T_all, h1[:], t)

    def ag(cc_in, cc_out):
        nc.gpsimd.collective_compute(
            "AllGather", mybir.AluOpType.bypass, replica_groups=GROUPS,
            ins=[cc_in.opt()], outs=[cc_out.opt()])

    # kT feature-major: [128 feats(head pair), T local keys] per ct
    def emit_k_quarter(cts, dst):
        for ct in cts:
            ps = mmpsA.tile([128, T], F32, tag="mmA")
            for fb in range(8):
                nc.tensor.matmul(
                    ps[:],
                    wk_sb[:, fb, ct * 128:(ct + 1) * 128],
                    h1T_all[:, fb, :],
                    start=(fb == 0), stop=(fb == 7),
                )
            nc.vector.tensor_scalar_add(dst[:, ct % 4, :], ps[:], bqki[:, 8 + ct:9 + ct])

    # v token-major, feature half cc: [128 tok, 512]
    def emit_v_half(cc, vloc, cc_v_in):
        for t in range(NT):
            ps = mmpsA.tile([128, 512], F32, tag="mmA")
            for fb in range(8):
                nc.tensor.matmul(
                    ps[:],
                    h1T_all[:, fb, t * 128:(t + 1) * 128],
                    wv_sb[:, fb, cc * 512:(cc + 1) * 512],
                    start=(fb == 0), stop=False,
                )
            nc.tensor.matmul(ps[:], ones_row[:], bvpo[:, cc * 512:(cc + 1) * 512],
                             start=False, stop=True)
            nc.scalar.copy(vloc[:, t, :], ps[:])
        nc.sync.dma_start(cc_v_in.rearrange("(t p) f -> p t f", p=128), vloc[:])

    def emit_q_quarter(cts):
        for ct in cts:
            ps = mmpsA.tile([128, T], F32, tag="mmA")
            for fb in range(8):
                nc.tensor.matmul(
                    ps[:],
                    wq_sb[:, fb, ct * 128:(ct + 1) * 128],
                    h1T_all[:, fb, :],
                    start=(fb == 0), stop=(fb == 7),
                )
            nc.vector.tensor_scalar_add(qT_all[:, ct, :], ps[:], bqki[:, ct:ct + 1])

    # interleave projections with gathers so the serial CC engine runs
    # K-A, V-A, K-B, V-B back-to-back, each starting as its input lands.
    emit_k_quarter(range(0, 4), ktA_sb)
    nc.sync.dma_start(cc_k_in_a.rearrange("(c p) k -> p c k", p=128), ktA_sb[:])
    ag(cc_k_in_a, cc_k_out_a)
    vloc_a = vlp.tile([128, NT, 512], BF16, tag="vloc", name="vloc_a")
    emit_v_half(0, vloc_a, cc_v_in_a)
    ag(cc_v_in_a, cc_v_out_a)
    emit_k_quarter(range(4, 8), ktB_sb)
    nc.sync.dma_start(cc_k_in_b.rearrange("(c p) k -> p c k", p=128), ktB_sb[:])
    ag(cc_k_in_b, cc_k_out_b)
    emit_q_quarter(range(0, 4))
    vloc_b = vlp.tile([128, NT, 512], BF16, tag="vloc", name="vloc_b")
    emit_v_half(1, vloc_b, cc_v_in_b)
    ag(cc_v_in_b, cc_v_out_b)
    emit_q_quarter(range(4, 8))

    nc.sync.dma_start(wproj_sb[:], io["wproj"].rearrange("(f p) h -> p f h", p=128))

    sA.close()

    # =====================================================================
    # Phase B: attention. scores^T per key-tile (row-packed head pairs),
    # exp on ACT, ctx^T via V'=[V|ones] (M=65), fast normalize at hp end.
    # =====================================================================
    sB = ExitStack()
    ktpool = sB.enter_context(tc.tile_pool(name="ktpool", bufs=5))
    vstg = sB.enter_context(tc.tile_pool(name="vstg", bufs=3))
    spool = sB.enter_context(tc.tile_pool(name="spool", bufs=2, space="PSUM"))
    cpool = sB.enter_context(tc.tile_pool(name="cpool", bufs=2, space="PSUM"))
    ppool = sB.enter_context(tc.tile_pool(name="ppool", bufs=8))
    rpool = sB.enter_context(tc.tile_pool(name="rpool", bufs=3))
    vsbp = sB.enter_context(tc.tile_pool(name="vsbp", bufs=1))
    vsb_a = vsbp.tile([128, NKT, 8, VW], BF16, name="vsb_a")
    vsb_b = vsbp.tile([128, NKT, 8, VW], BF16, name="vsb_b")

    # ones columns for all key tiles (DVE; no data deps)
    nc.vector.memset(vsb_a[:, :, :, DH:VW], 1.0)
    nc.vector.memset(vsb_b[:, :, :, DH:VW], 1.0)

    def load_kt(hp):
        """kT for head pair hp: [128 feats, 4 rank-blocks, 512 keys]."""
        kt = ktpool.tile([128, G, T], BF16, tag="kt", name=f"kt{hp}")
        cko, hpo = (cc_k_out_a, hp) if hp < 4 else (cc_k_out_b, hp - 4)
        src = cko.rearrange("(g c p) k -> c p g k", g=G, c=4, p=128)[hpo]
        nc.sync.dma_start(kt[:], src)
        return kt

    def load_v_half(vsb, cc_v_out):
        """Interleave gathered V [tokens, 512] into vsb [128, kt, head, VW]."""
        for g in range(G):
            vplain = vstg.tile([128, NT, 512], BF16, tag="vplain", name=f"vp{g}")
            src = cc_v_out.rearrange("(g t p) f -> g p t f", g=G, p=128)[g]
            nc.sync.dma_start(vplain[:], src)
            for t in range(NT):
                kt = g * 4 + t
                nc.vector.tensor_copy(
                    vsb[:, kt, :, 0:DH],
                    vplain[:, t, :].rearrange("p (h d) -> p h d", d=DH),
                )

    def attend_hp(hp, kt_hp, vsb):
        hh = (hp % 4) * 2  # head-within-half base index
        cps0 = cpool.tile([128, T], F32, tag="ctx0")
        cps1 = cpool.tile([128, T], F32, tag="ctx1")

        def emit_ctx(kt, pb, first, last):
            for h, cps in enumerate((cps0, cps1)):
                nc.tensor.matmul(
                    cps[0:VW, :],
                    vsb[:, kt, hh + h, :],
                    pb[:, h * T:(h + 1) * T],
                    start=first, stop=last,
                )

        # software-pipelined: scores(kt) | ctx(kt-1) | exp(kt)
        prev = None
        for kt in range(NKT):
            g, j = kt // 4, kt % 4
            ps = spool.tile([128, 1024], F32, tag="ps", name="ps")
            nc.tensor.matmul(
                ps[:, 0:T],
                kt_hp[0:64, g, j * 128:(j + 1) * 128],
                qT_all[0:64, hp, :],
                start=True, stop=True, tile_position=(0, 0),
            )
            nc.tensor.matmul(
                ps[:, T:1024],
                kt_hp[64:128, g, j * 128:(j + 1) * 128],
                qT_all[64:128, hp, :],
                start=True, stop=True, tile_position=(64, 0),
            )
            if prev is not None:
                emit_ctx(prev[0], prev[1], prev[2], False)
            pb = ppool.tile([128, 1024], BF16, tag="pb", name="pb")
            nc.scalar.activation(pb[:], ps[:], AF.Exp)
            prev = (kt, pb, kt == 0)
        emit_ctx(prev[0], prev[1], prev[2], True)

        # normalize: 1/sumexp (row 64), broadcast across 64 partitions via a
        # K=1 matmul into the same tile's spare rows 64:128 (no extra PSUM
        # bank), then one DVE multiply into ctxT.
        for h, cps in enumerate((cps0, cps1)):
            rc = rpool.tile([1, T], F32, tag=f"rc{h}")
            rcb = rpool.tile([1, T], BF16, tag=f"rcb{h}")
            rbs = rpool.tile([64, T], F32, tag=f"rbs{h}")
            nc.vector.reciprocal(rc[:], cps[DH:VW, :])
            nc.vector.tensor_copy(rcb[:], rc[:])
            nc.tensor.matmul(cps[64:128, :], ones_row[:, 0:64], rcb[:],
                             start=True, stop=True, tile_position=(0, 64))
            nc.vector.tensor_copy(rbs[:], cps[64:128, :])
            nc.vector.tensor_mul(
                ctxT_all[h * 64:(h + 1) * 64, hp * T:(hp + 1) * T],
                cps[0:DH, :], rbs[:])

    # half A
    kt_tiles = {}
    kt_tiles[0] = load_kt(0)
    kt_tiles[1] = load_kt(1)
    load_v_half(vsb_a, cc_v_out_a)
    kt_tiles[2] = load_kt(2)
    kt_tiles[3] = load_kt(3)
    for hp in range(4):
        attend_hp(hp, kt_tiles[hp], vsb_a)
    # half B
    kt_tiles[4] = load_kt(4)
    load_v_half(vsb_b, cc_v_out_b)
    for hp in range(4, NHP):
        if hp + 1 < NHP:
            kt_tiles[hp + 1] = load_kt(hp + 1)
        attend_hp(hp, kt_tiles[hp], vsb_b)

    sB.close()

    # =====================================================================
    # Phase C: proj (token-major) + residual -> x2, LN2 -> h2T
    # =====================================================================
    sC = ExitStack()
    lnp2 = sC.enter_context(tc.tile_pool(name="lnp2", bufs=2))
    h2p = sC.enter_context(tc.tile_pool(name="h2p", bufs=2))
    mmpsC = sC.enter_context(tc.tile_pool(name="mmpsC", bufs=3, space="PSUM"))
    tpsC = None
    stgC = None
    if not USE_DMA_TRANSPOSE:
        tpsC = sC.enter_context(tc.tile_pool(name="tpsC", bufs=2, space="PSUM"))
        stgC = sC.enter_context(tc.tile_pool(name="stgC", bufs=2))

    for t in range(NT):
        for cc in range(2):
            ps = mmpsC.tile([128, 512], F32, tag="pj")
            for hp in range(8):
                nc.tensor.matmul(
                    ps[:],
                    ctxT_all[:, hp * T + t * 128: hp * T + (t + 1) * 128],
                    wproj_sb[:, hp, cc * 512:(cc + 1) * 512],
                    start=(hp == 0), stop=False,
                )
            nc.tensor.matmul(ps[:], ones_row[:], bvpo[:, H + cc * 512:H + (cc + 1) * 512],
                             start=False, stop=True)
            nc.vector.tensor_add(
                x2_all[:, t, cc * 512:(cc + 1) * 512],
                ps[:], x_all[:, t, cc * 512:(cc + 1) * 512])
        rs, nmr = layer_norm_stats(lnp2, x2_all[:, t, :])
        h2 = h2p.tile([128, H], BF16, tag="h2")
        nc.scalar.activation(h2[:], x2_all[:, t, :], AF.Identity, bias=nmr[:], scale=rs[:])
        if USE_DMA_TRANSPOSE:
            dst = h2T_all.rearrange("p f (tt c) -> p f tt c", c=128)[:, :, t, :]
            nc.scalar.dma_start_transpose(dst, h2[:])
        else:
            dst = h2T_all.rearrange("p f (tt c) -> p f tt c", c=128)[:, :, t, :]
            for fb in range(8):
                pst = tpsC.tile([128, 128], BF16, tag="tp")
                nc.tensor.transpose(pst[:], h2[:, fb * 128:(fb + 1) * 128], ident[:])
                nc.vector.tensor_copy(dst[:, fb, :], pst[:])

    sC.close()

    # =====================================================================
    # Phase D+E fused: per g: wi matmuls + gelu -> h3T[g], then wo matmuls
    # for output columns 0:512 accumulate into 4 persistent psums.
    # Second pass re-reads h3T for output columns 512:1024.
    # Weight streams ride the gpsimd SWDGE queue (batched loads).
    # =====================================================================
    sD = ExitStack()
    h3p = sD.enter_context(tc.tile_pool(name="h3p", bufs=1))
    wip = sD.enter_context(tc.tile_pool(name="wip", bufs=3))
    wop = sD.enter_context(tc.tile_pool(name="wop", bufs=2))
    mmpsD = sD.enter_context(tc.tile_pool(name="mmpsD", bufs=3, space="PSUM"))
    wops = sD.enter_context(tc.tile_pool(name="wops", bufs=1, space="PSUM"))
    outp = sD.enter_context(tc.tile_pool(name="outp", bufs=2))

    h3T_all = h3p.tile([128, NG, T], BF16, name="h3T_all")

    WIB = 4   # wi groups per DMA
    WOB = 8   # wo groups per DMA

    wi_tiles = [None] * (NG // WIB)
    wo_tiles = {}

    def load_wi(b):
        wi = wip.tile([128, WIB, 8, 128], BF16, tag="wi", name=f"wi{b}")
        src = io["wi"][b * WIB:(b + 1) * WIB].rearrange("g p f c -> p g f c")
        nc.gpsimd.dma_start(wi[:], src)
        return wi

    def load_wo(cc, b):
        wo = wop.tile([128, WOB, 512], BF16, tag="wo", name=f"wo{cc}_{b}")
        src = io["wo"][:, cc * 512:(cc + 1) * 512].rearrange(
            "(bb g p) f -> bb p g f", g=WOB, p=128)[b]
        nc.gpsimd.dma_start(wo[:], src)
        return wo

    # prefetch schedule on the gpsimd queue (wi bufs=3, wo bufs=2 gate it)
    wi_tiles[0] = load_wi(0)
    wi_tiles[1] = load_wi(1)
    wo_tiles[(0, 0)] = load_wo(0, 0)
    wi_tiles[2] = load_wi(2)
    wo_tiles[(0, 1)] = load_wo(0, 1)

    psE = [wops.tile([128, 512], F32, tag=f"wo_ps{t}", name=f"wo_ps{t}") for t in range(NT)]
    for g in range(NG):
        b, gi = g // WIB, g % WIB
        if b + 3 < len(wi_tiles) and wi_tiles[b + 3] is None and gi == 0:
            wi_tiles[b + 3] = load_wi(b + 3)
        wob = g // WOB
        if gi == 1 and b % 2 == 1 and (0, b // 2 + 2) not in wo_tiles and b // 2 + 2 < 4:
            wo_tiles[(0, b // 2 + 2)] = load_wo(0, b // 2 + 2)
        wi = wi_tiles[b]
        ps = mmpsD.tile([128, T], F32, tag="wi_ps", name="wi_ps")
        for fb in range(8):
            nc.tensor.matmul(
                ps[:], wi[:, gi, fb, :], h2T_all[:, fb, :],
                start=(fb == 0), stop=(fb == 7),
            )
        nc.scalar.activation(h3T_all[:, g, :], ps[:],
                             AF.Gelu_apprx_tanh, bias=bqki[:, 16 + g:17 + g])
        wo = wo_tiles[(0, wob)]
        for t in range(NT):
            nc.tensor.matmul(
                psE[t][:],
                h3T_all[:, g, t * 128:(t + 1) * 128],
                wo[:, g % WOB, :],
                start=(g == 0), stop=False,
            )
    ostage = outp.tile([128, NT, 512], F32, tag="ostage", name="ostage0")
    for t in range(NT):
        nc.tensor.matmul(psE[t][:], ones_row[:], bvpo[:, 2 * H:2 * H + 512],
                         start=False, stop=True)
        nc.vector.tensor_add(ostage[:, t, :], psE[t][:], x2_all[:, t, 0:512])
    nc.sync.dma_start(
        out_d[:, 0:512].rearrange("(t p) f -> p t f", p=128), ostage[:])

    # second pass: output columns 512:1024
    wo_tiles[(1, 0)] = load_wo(1, 0)
    wo_tiles[(1, 1)] = load_wo(1, 1)
    psE2 = [wops.tile([128, 512], F32, tag=f"wo_ps{t}", name=f"wo2_ps{t}") for t in range(NT)]
    for g in range(NG):
        wob = g // WOB
        if g % WOB == 1 and (1, wob + 2) not in wo_tiles and wob + 2 < 4:
            wo_tiles[(1, wob + 2)] = load_wo(1, wob + 2)
        wo = wo_tiles[(1, wob)]
        for t in range(NT):
            nc.tensor.matmul(
                psE2[t][:],
                h3T_all[:, g, t * 128:(t + 1) * 128],
                wo[:, g % WOB, :],
                start=(g == 0), stop=False,
            )
    ostage2 = outp.tile([128, NT, 512], F32, tag="ostage", name="ostage1")
    for t in range(NT):
        nc.tensor.matmul(psE2[t][:], ones_row[:], bvpo[:, 2 * H + 512:3 * H],
                         start=False, stop=True)
        nc.vector.tensor_add(ostage2[:, t, :], psE2[t][:], x2_all[:, t, 512:1024])
    nc.sync.dma_start(
        out_d[:, 512:1024].rearrange("(t p) f -> p t f", p=128), ostage2[:])

    sD.close()
    s_outer.close()


def _build_program():
    nc = bass.Bass("TRN2", target_bir_lowering=False, debug=False, num_devices=NC)
    io = {}
    io["x"] = nc.dram_tensor("x", [T, H], F32, kind="ExternalInput").ap()
    io["wq"] = nc.dram_tensor("wq", [H, H], BF16, kind="ExternalInput").ap()
    io["wk"] = nc.dram_tensor("wk", [H, H], BF16, kind="ExternalInput").ap()
    io["wv"] = nc.dram_tensor("wv", [H, H], BF16, kind="ExternalInput").ap()
    io["wproj"] = nc.dram_tensor("wproj", [H, H], BF16, kind="ExternalInput").ap()
    io["wi"] = nc.dram_tensor("wi", [NG, 128, 8, 128], BF16, kind="ExternalInput").ap()
    io["wo"] = nc.dram_tensor("wo", [FFN, H], BF16, kind="ExternalInput").ap()
    io["bqki"] = nc.dram_tensor("bqki", [128, 48], F32, kind="ExternalInput").ap()
    io["bvpo"] = nc.dram_tensor("bvpo", [1, 3 * H], BF16, kind="ExternalInput").ap()
    io["ident"] = nc.dram_tensor("ident", [128, 128], BF16, kind="ExternalInput").ap()
    io["ones_row"] = nc.dram_tensor("ones_row", [1, 128], BF16, kind="ExternalInput").ap()
    io["out"] = nc.dram_tensor("out", [T, H], F32, kind="ExternalOutput").ap()
    with tile.TileContext(nc) as tc:
        _emit(tc, nc, io)
    _split_multiwait(nc)
    return nc


_PROGRAM = None
LAST_RESULTS = None


def kernel(x, ln1_scale, ln1_bias, qkv_w, qkv_b, proj_w, proj_b,
           ln2_scale, ln2_bias, wi_w, wi_b, wo_w, wo_b):
    global _PROGRAM, LAST_RESULTS
    x = np.asarray(x, np.float32)
    ln1_scale = np.asarray(ln1_scale, np.float32); ln1_bias = np.asarray(ln1_bias, np.float32)
    qkv_w = np.asarray(qkv_w, np.float32); qkv_b = np.asarray(qkv_b, np.float32)
    proj_w = np.asarray(proj_w, np.float32); proj_b = np.asarray(proj_b, np.float32)
    ln2_scale = np.asarray(ln2_scale, np.float32); ln2_bias = np.asarray(ln2_bias, np.float32)
    wi_w = np.asarray(wi_w, np.float32); wi_b = np.asarray(wi_b, np.float32)
    wo_w = np.asarray(wo_w, np.float32); wo_b = np.asarray(wo_b, np.float32)

    # fold LN affine params into the next matmul's weights/biases
    qkv_w_eff = ln1_scale[:, None] * qkv_w
    qkv_b_eff = qkv_b + ln1_bias @ qkv_w
    w3 = qkv_w_eff.reshape(H, NH, 3, DH)
    b3 = qkv_b_eff.reshape(NH, 3, DH)
    scale = 1.0 / np.sqrt(np.float32(DH))
    wq = (w3[:, :, 0, :] * scale).reshape(H, H)
    wk = w3[:, :, 1, :].reshape(H, H)
    wv = w3[:, :, 2, :].reshape(H, H)
    bq_v = (b3[:, 0, :] * scale).reshape(H)
    bk_v = b3[:, 1, :].reshape(H)
    bv_v = b3[:, 2, :].reshape(H)
    wi_eff = ln2_scale[:, None] * wi_w
    bi_v = wi_b + ln2_bias @ wi_w

    bqki = np.concatenate([
        bq_v.reshape(8, 128).T, bk_v.reshape(8, 128).T,
        bi_v.reshape(32, 128).T], axis=1).astype(np.float32)
    bvpo = np.concatenate([bv_v, proj_b, wo_b]).reshape(1, 3 * H)

    common = {
        "wq": wq.astype(NPBF16), "wk": wk.astype(NPBF16), "wv": wv.astype(NPBF16),
        "wproj": proj_w.astype(NPBF16),
        "wi": np.ascontiguousarray(
            wi_eff.astype(NPBF16).reshape(8, 128, 32, 128).transpose(2, 1, 0, 3)),
        "wo": wo_w.astype(NPBF16),
        "bqki": np.ascontiguousarray(bqki),
        "bvpo": bvpo.astype(NPBF16),
        "ident": np.eye(128, dtype=NPBF16),
        "ones_row": np.ones((1, 128), NPBF16),
    }
    x_flat = x.reshape(B * S, H)
    in_maps = []
    for c in range(NC):
        m = dict(common)
        m["x"] = np.ascontiguousarray(x_flat[c * T:(c + 1) * T, :])
        in_maps.append(m)

    if _PROGRAM is None:
        _PROGRAM = _build_program()
    r = run_bass_kernel_spmd(_PROGRAM, in_maps, list(range(NC)))
    LAST_RESULTS = r
    out = np.concatenate([r.results[c]["out"] for c in range(NC)], axis=0)
    return out.reshape(B, S, H).astype(np.float32)


# revision 10
# speedup vs baseline: 1.2112x; 1.1551x over previous
"""Trainium2 Bass kernel for a pre-LN transformer block (B=2,S=2048,H=1024,NH=16,FFN=4096).

Sharding: 8 cores, 512 tokens/core (4 cores per batch element). K/V are
exchanged within each batch group via four 4-rank AllGathers (K and V each in
two head-halves) so attention on early head-pairs overlaps the later gathers.
All matmuls run in bf16 on the PE array with fp32 PSUM accumulation; LayerNorm
statistics, residuals and the final output stay fp32.

Self-contained: hardcodes shapes; builds the Bass program once and runs it via
run_bass_kernel_spmd on cores 0-7.
"""

import sys

for _p in ("/root/.axon_site/_ro/trn_rl_repo", "/opt/trn_rl_repo"):
    if _p not in sys.path:
        sys.path.append(_p)

import numpy as np
import ml_dtypes

# If BASS_TRACE is set but the axon NTFF hook module is missing, the trace
# path would crash on import; pre-register a no-op hook shim so tracing
# degrades gracefully instead.
try:
    import antenv.axon_hooks  # noqa: F401
except ImportError:
    import types as _types
    _m = _types.ModuleType("antenv.axon_hooks")
    _m._hook = None
    _m.get_axon_ntff_profile_hook = lambda: _m._hook
    _m.set_axon_ntff_profile_hook = lambda h: setattr(_m, "_hook", h)
    sys.modules["antenv.axon_hooks"] = _m

import bass_rust
import concourse.bass as bass
import concourse.mybir as mybir
import concourse.tile as tile
from concourse.bass_utils import run_bass_kernel_spmd

BF16 = mybir.dt.bfloat16
F32 = mybir.dt.float32
AF = mybir.ActivationFunctionType
NPBF16 = np.dtype(ml_dtypes.bfloat16)

B, S, H, NH, DH, FFN = 2, 2048, 1024, 16, 64, 4096
NC = 8                      # cores
T = 512                     # tokens per core
NT = T // 128               # token tiles per core (4)
GROUPS = [[0, 1, 2, 3], [4, 5, 6, 7]]
G = 4                       # cores per batch group
NKT = 16                    # key tiles per batch (4 ranks x 4)
NHP = NH // 2               # head pairs (8)
EPS = 1e-3
VW = DH + 1                 # 65: V columns + ones column per head
NG = FFN // 128             # 32 ffn row-groups

USE_DMA_TRANSPOSE = False   # xbar DMA transpose serializes ~6us/tile; PE wins

# ---------------------------------------------------------------------------
# Workaround: this walrus build rejects >1 inline sync-wait per instruction.
# After Tile scheduling, move excess waits onto single-wait NoOp carriers
# inserted immediately before the over-limit instruction (same engine, same
# block, so per-engine program order and wait semantics are preserved).
# ---------------------------------------------------------------------------
def _split_multiwait(nc, limit=1):
    n_new = 0
    for f in nc.m.functions:
        for blk in f.blocks:
            insts = blk.instructions
            out = []
            for ins in insts:
                si = getattr(ins, "sync_info", None)
                waits = list(si.on_wait) if si is not None else []
                if len(waits) > limit:
                    for i, w in enumerate(waits[:-limit]):
                        nop = mybir.InstNoOp(
                            name=f"{ins.name}_w{i}",
                            sync_info=mybir.SyncInfo(on_wait=[w], on_update=[]),
                            bass_nofuse=True,
                            engine=ins.engine,
                        )
                        out.append(nop)
                        n_new += 1
                    ins.sync_info = mybir.SyncInfo(
                        on_wait=waits[-limit:], on_update=list(si.on_update)
                    )
                out.append(ins)
            if len(out) != len(insts):
                blk.instructions = out
    return n_new


def _emit(tc, nc, io):
    """Emit the per-core program. io: dict of DRAM APs."""
    from contextlib import ExitStack

    x_d = io["x"]
    out_d = io["out"]

    s_outer = ExitStack()

    constp = s_outer.enter_context(tc.tile_pool(name="constp", bufs=1))
    dramp = s_outer.enter_context(tc.tile_pool(name="dramp", bufs=1, space="DRAM"))

    # ---- phase-A bulk loads, one descriptor-dense DMA each (SP queue) ----
    persp = s_outer.enter_context(tc.tile_pool(name="persp", bufs=1))
    x_all = persp.tile([128, NT, H], F32, name="x_all")
    nc.sync.dma_start(x_all[:], x_d.rearrange("(t p) h -> p t h", p=128))

    sA = ExitStack()
    wpool = sA.enter_context(tc.tile_pool(name="wpool", bufs=3))
    wk_sb = wpool.tile([128, 8, H], BF16, tag="w3", name="wk_sb")
    nc.sync.dma_start(wk_sb[:], io["wk"].rearrange("(f p) h -> p f h", p=128))

    # constants / biases
    ident = constp.tile([128, 128], BF16)
    nc.sync.dma_start(ident[:], io["ident"][:])
    ones_row = constp.tile([1, 128], BF16)
    nc.sync.dma_start(ones_row[:], io["ones_row"][:])
    # bqki: cols 0:8 = bq, 8:16 = bk, 16:48 = bi (all [128, n])
    bqki = constp.tile([128, 48], F32)
    nc.sync.dma_start(bqki[:], io["bqki"][:])
    # bvpo: [1, 3H] bf16: bv | bproj | bo
    bvpo = constp.tile([1, 3 * H], BF16)
    nc.sync.dma_start(bvpo[:], io["bvpo"][:])
    eps_t = constp.tile([128, 1], F32)
    nc.gpsimd.memset(eps_t[:], float(EPS))

    # Warm-up collective: the first collective on a NEFF pays a ~60us CC
    # engine init latency. Pay it at t=0 on a dummy 1KB gather (no input
    # deps) so the real K/V gathers start promptly.
    cc_warm_in = dramp.tile([4, 128], BF16)
    cc_warm_out = dramp.tile([16, 128], BF16)
    nc.gpsimd.collective_compute(
        "AllGather", mybir.AluOpType.bypass, replica_groups=GROUPS,
        ins=[cc_warm_in.opt()], outs=[cc_warm_out.opt()])

    wq_sb = wpool.tile([128, 8, H], BF16, tag="w3", name="wq_sb")
    nc.sync.dma_start(wq_sb[:], io["wq"].rearrange("(f p) h -> p f h", p=128))
    wv_sb = wpool.tile([128, 8, H], BF16, tag="w3", name="wv_sb")
    nc.sync.dma_start(wv_sb[:], io["wv"].rearrange("(f p) h -> p f h", p=128))

    # persistent activations
    x2_all = persp.tile([128, NT, H], F32, name="x2_all")
    qT_all = persp.tile([128, 8, T], BF16, name="qT_all")
    ctxT_all = persp.tile([128, 8 * T], BF16, name="ctxT_all")
    h2T_all = persp.tile([128, 8, T], BF16, name="h2T_all")
    wproj_sb = persp.tile([128, 8, H], BF16, name="wproj_sb")

    # collective buffers (AllGather concatenates along dim 0)
    cc_k_in_a = dramp.tile([512, T], BF16)     # kT ct0-3 (heads 0-7)
    cc_k_in_b = dramp.tile([512, T], BF16)     # kT ct4-7 (heads 8-15)
    cc_k_out_a = dramp.tile([G * 512, T], BF16)
    cc_k_out_b = dramp.tile([G * 512, T], BF16)
    cc_v_in_a = dramp.tile([T, 512], BF16)     # v cols 0:512 (heads 0-7)
    cc_v_in_b = dramp.tile([T, 512], BF16)     # v cols 512:1024 (heads 8-15)
    cc_v_out_a = dramp.tile([G * T, 512], BF16)
    cc_v_out_b = dramp.tile([G * T, 512], BF16)

    def layer_norm_stats(pool, x_slice):
        """x_slice [128,H] f32 -> (rs [128,1], nmr [128,1]) in SBUF."""
        stats = pool.tile([128, 2, 6], F32, tag="ln_st")
        mv = pool.tile([128, 2], F32, tag="ln_mv")
        std = pool.tile([128, 1], F32, tag="ln_std")
        rs = pool.tile([128, 1], F32, tag="ln_rs")
        nmr = pool.tile([128, 1], F32, tag="ln_nmr")
        xc = x_slice.rearrange("p (n c) -> p n c", c=512)
        nc.vector.bn_stats(out=stats[:, 0, :], in_=xc[:, 0, :])
        nc.vector.bn_stats(out=stats[:, 1, :], in_=xc[:, 1, :])
        nc.vector.bn_aggr(out=mv[:], in_=stats[:])
        nc.scalar.activation(std[:], mv[:, 1:2], AF.Sqrt, bias=eps_t[:])
        nc.vector.reciprocal(rs[:], std[:])
        nc.vector.tensor_mul(nmr[:], mv[:, 0:1], rs[:])
        nc.vector.tensor_scalar_mul(nmr[:], nmr[:], -1.0)
        return rs, nmr

    # =====================================================================
    # Phase A: load x, LN1 -> h1, h1T (xbar transpose), kT, v, qT.
    # K/V AllGathers (by head-half) dispatched as soon as inputs land.
    # =====================================================================
    lnp = sA.enter_context(tc.tile_pool(name="lnp", bufs=2))
    h1p = sA.enter_context(tc.tile_pool(name="h1p", bufs=2))
    h1Tp = sA.enter_context(tc.tile_pool(name="h1Tp", bufs=1))
    ktp = sA.enter_context(tc.tile_pool(name="ktp", bufs=1))
    vlp = sA.enter_context(tc.tile_pool(name="vlp", bufs=2))
    mmpsA = sA.enter_context(tc.tile_pool(name="mmpsA", bufs=3, space="PSUM"))
    tpsA = None
    stgA = None
    if not USE_DMA_TRANSPOSE:
        tpsA = sA.enter_context(tc.tile_pool(name="tpsA", bufs=2, space="PSUM"))
        stgA = sA.enter_context(tc.tile_pool(name="stgA", bufs=2))

    h1T_all = h1Tp.tile([128, 8, T], BF16, name="h1T_all")
    ktA_sb = ktp.tile([128, 4, T], BF16, name="ktA_sb")
    ktB_sb = ktp.tile([128, 4, T], BF16, name="ktB_sb")

    def transpose_tile(dstT_all, h_tile, t):
        """h_tile [128, H] bf16 -> dstT_all[:, fb, t*128:(t+1)*128] for all fb."""
        dst = dstT_all.rearrange("p f (tt c) -> p f tt c", c=128)[:, :, t, :]
        if USE_DMA_TRANSPOSE:
            nc.scalar.dma_start_transpose(dst, h_tile)
        else:
            for fb in range(8):
                ps = tpsA.tile([128, 128], BF16, tag="tp")
                nc.tensor.transpose(ps[:], h_tile[:, fb * 128:(fb + 1) * 128], ident[:])
                nc.vector.tensor_copy(dst[:, fb, :], ps[:])

    for t in range(NT):
        rs, nmr = layer_norm_stats(lnp, x_all[:, t, :])
        h1 = h1p.tile([128, H], BF16, tag="h1")
        nc.scalar.activation(h1[:], x_all[:, t, :], AF.Identity, bias=nmr[:], scale=rs[:])
        transpose_tile(h1T_all, h1[:], t)

    def ag(cc_in, cc_out):
        nc.gpsimd.collective_compute(
            "AllGather", mybir.AluOpType.bypass, replica_groups=GROUPS,
            ins=[cc_in.opt()], outs=[cc_out.opt()])

    # kT feature-major: [128 feats(head pair), T local keys] per ct
    def emit_k_quarter(cts, dst):
        for ct in cts:
            ps = mmpsA.tile([128, T], F32, tag="mmA")
            for fb in range(8):
                nc.tensor.matmul(
                    ps[:],
                    wk_sb[:, fb, ct * 128:(ct + 1) * 128],
                    h1T_all[:, fb, :],
                    start=(fb == 0), stop=(fb == 7),
                )
            nc.vector.tensor_scalar_add(dst[:, ct % 4, :], ps[:], bqki[:, 8 + ct:9 + ct])

    # v token-major, feature half cc: [128 tok, 512]
    def emit_v_half(cc, vloc, cc_v_in):
        for t in range(NT):
            ps = mmpsA.tile([128, 512], F32, tag="mmA")
            for fb in range(8):
                nc.tensor.matmul(
                    ps[:],
                    h1T_all[:, fb, t * 128:(t + 1) * 128],
                    wv_sb[:, fb, cc * 512:(cc + 1) * 512],
                    start=(fb == 0), stop=False,
                )
            nc.tensor.matmul(ps[:], ones_row[:], bvpo[:, cc * 512:(cc + 1) * 512],
                             start=False, stop=True)
            nc.scalar.copy(vloc[:, t, :], ps[:])
        nc.sync.dma_start(cc_v_in.rearrange("(t p) f -> p t f", p=128), vloc[:])

    def emit_q_quarter(cts):
        for ct in cts:
            ps = mmpsA.tile([128, T], F32, tag="mmA")
            for fb in range(8):
                nc.tensor.matmul(
                    ps[:],
                    wq_sb[:, fb, ct * 128:(ct + 1) * 128],
                    h1T_all[:, fb, :],
                    start=(fb == 0), stop=(fb == 7),
                )
            nc.vector.tensor_scalar_add(qT_all[:, ct, :], ps[:], bqki[:, ct:ct + 1])

    # interleave projections with gathers so the serial CC engine runs
    # K-A, V-A, K-B, V-B back-to-back, each starting as its input lands.
    emit_k_quarter(range(0, 4), ktA_sb)
    nc.sync.dma_start(cc_k_in_a.rearrange("(c p) k -> p c k", p=128), ktA_sb[:])
    ag(cc_k_in_a, cc_k_out_a)
    vloc_a = vlp.tile([128, NT, 512], BF16, tag="vloc", name="vloc_a")
    emit_v_half(0, vloc_a, cc_v_in_a)
    ag(cc_v_in_a, cc_v_out_a)
    emit_k_quarter(range(4, 8), ktB_sb)
    nc.sync.dma_start(cc_k_in_b.rearrange("(c p) k -> p c k", p=128), ktB_sb[:])
    ag(cc_k_in_b, cc_k_out_b)
    emit_q_quarter(range(0, 4))
    vloc_b = vlp.tile([128, NT, 512], BF16, tag="vloc", name="vloc_b")
    emit_v_half(1, vloc_b, cc_v_in_b)
    ag(cc_v_in_b, cc_v_out_b)
    emit_q_quarter(range(4, 8))

    nc.sync.dma_start(wproj_sb[:], io["wproj"].rearrange("(f p) h -> p f h", p=128))

    sA.close()

    # =====================================================================
    # Phase B: attention. scores^T per key-tile (row-packed head pairs),
    # exp on ACT, ctx^T via V'=[V|ones] (M=65), fast normalize at hp end.
    # =====================================================================
    sB = ExitStack()
    ktpool = sB.enter_context(tc.tile_pool(name="ktpool", bufs=5))
    vstg = sB.enter_context(tc.tile_pool(name="vstg", bufs=3))
    spool = sB.enter_context(tc.tile_pool(name="spool", bufs=2, space="PSUM"))
    cpool = sB.enter_context(tc.tile_pool(name="cpool", bufs=2, space="PSUM"))
    ppool = sB.enter_context(tc.tile_pool(name="ppool", bufs=8))
    rpool = sB.enter_context(tc.tile_pool(name="rpool", bufs=3))
    vsbp = sB.enter_context(tc.tile_pool(name="vsbp", bufs=1))
    vsb_a = vsbp.tile([128, NKT, 8, VW], BF16, name="vsb_a")
    vsb_b = vsbp.tile([128, NKT, 8, VW], BF16, name="vsb_b")

    # ones columns for all key tiles (DVE; no data deps)
    nc.vector.memset(vsb_a[:, :, :, DH:VW], 1.0)
    nc.vector.memset(vsb_b[:, :, :, DH:VW], 1.0)

    def load_kt(hp):
        """kT for head pair hp: [128 feats, 4 rank-blocks, 512 keys]."""
        kt = ktpool.tile([128, G, T], BF16, tag="kt", name=f"kt{hp}")
        cko, hpo = (cc_k_out_a, hp) if hp < 4 else (cc_k_out_b, hp - 4)
        src = cko.rearrange("(g c p) k -> c p g k", g=G, c=4, p=128)[hpo]
        nc.sync.dma_start(kt[:], src)
        return kt

    def load_v_half(vsb, cc_v_out):
        """Interleave gathered V [tokens, 512] into vsb [128, kt, head, VW]."""
        for g in range(G):
            vplain = vstg.tile([128, NT, 512], BF16, tag="vplain", name=f"vp{g}")
            src = cc_v_out.rearrange("(g t p) f -> g p t f", g=G, p=128)[g]
            nc.sync.dma_start(vplain[:], src)
            for t in range(NT):
                kt = g * 4 + t
                nc.vector.tensor_copy(
                    vsb[:, kt, :, 0:DH],
                    vplain[:, t, :].rearrange("p (h d) -> p h d", d=DH),
                )

    def attend_hp(hp, kt_hp, vsb, pending_tail):
        """Returns a closure finishing this hp's softmax-normalize; the PE
        part of the previous hp's tail is emitted mid-loop (at kt==3) so
        the PE queue never stalls waiting on the DVE reciprocal chain."""
        hh = (hp % 4) * 2  # head-within-half base index
        cps0 = cpool.tile([128, T], F32, tag="ctx0")
        cps1 = cpool.tile([128, T], F32, tag="ctx1")

        def emit_ctx(kt, pb, first, last):
            for h, cps in enumerate((cps0, cps1)):
                nc.tensor.matmul(
                    cps[0:VW, :],
                    vsb[:, kt, hh + h, :],
                    pb[:, h * T:(h + 1) * T],
                    start=first, stop=last,
                )

        # software-pipelined: scores(kt) | ctx(kt-1) | exp(kt)
        prev = None
        for kt in range(NKT):
            g, j = kt // 4, kt % 4
            if kt == 3 and pending_tail is not None:
                pending_tail()
                pending_tail = None
            ps = spool.tile([128, 1024], F32, tag="ps", name="ps")
            nc.tensor.matmul(
                ps[:, 0:T],
                kt_hp[0:64, g, j * 128:(j + 1) * 128],
                qT_all[0:64, hp, :],
                start=True, stop=True, tile_position=(0, 0),
            )
            nc.tensor.matmul(
                ps[:, T:1024],
                kt_hp[64:128, g, j * 128:(j + 1) * 128],
                qT_all[64:128, hp, :],
                start=True, stop=True, tile_position=(64, 0),
            )
            if prev is not None:
                emit_ctx(prev[0], prev[1], prev[2], False)
            pb = ppool.tile([128, 1024], BF16, tag="pb", name="pb")
            nc.scalar.activation(pb[:], ps[:], AF.Exp)
            prev = (kt, pb, kt == 0)
        emit_ctx(prev[0], prev[1], prev[2], True)

        # normalize: 1/sumexp (row 64) on DVE now; broadcast via a K=1
        # matmul into the same tile's spare rows 64:128 (no extra PSUM
        # bank) deferred into the next hp's score stream.
        tails = []
        for h, cps in enumerate((cps0, cps1)):
            rc = rpool.tile([1, T], F32, tag=f"rc{h}")
            rcb = rpool.tile([1, T], BF16, tag=f"rcb{h}")
            nc.vector.reciprocal(rc[:], cps[DH:VW, :])
            nc.vector.tensor_copy(rcb[:], rc[:])
            tails.append((h, cps, rcb))

        def tail():
            for h, cps, rcb in tails:
                rbs = rpool.tile([64, T], F32, tag=f"rbs{h}")
                nc.tensor.matmul(cps[64:128, :], ones_row[:, 0:64], rcb[:],
                                 start=True, stop=True, tile_position=(0, 64))
                nc.vector.tensor_copy(rbs[:], cps[64:128, :])
                nc.vector.tensor_mul(
                    ctxT_all[h * 64:(h + 1) * 64, hp * T:(hp + 1) * T],
                    cps[0:DH, :], rbs[:])
        return tail

    # half A
    kt_tiles = {}
    kt_tiles[0] = load_kt(0)
    kt_tiles[1] = load_kt(1)
    load_v_half(vsb_a, cc_v_out_a)
    kt_tiles[2] = load_kt(2)
    kt_tiles[3] = load_kt(3)
    tail = None
    for hp in range(4):
        tail = attend_hp(hp, kt_tiles[hp], vsb_a, tail)
    # half B
    kt_tiles[4] = load_kt(4)
    load_v_half(vsb_b, cc_v_out_b)
    for hp in range(4, NHP):
        if hp + 1 < NHP:
            kt_tiles[hp + 1] = load_kt(hp + 1)
        tail = attend_hp(hp, kt_tiles[hp], vsb_b, tail)
    tail()  # last hp's normalize, right before proj consumes ctxT

    sB.close()

    # =====================================================================
    # Phase C: proj (token-major) + residual -> x2, LN2 -> h2T
    # =====================================================================
    sC = ExitStack()
    lnp2 = sC.enter_context(tc.tile_pool(name="lnp2", bufs=2))
    h2p = sC.enter_context(tc.tile_pool(name="h2p", bufs=2))
    mmpsC = sC.enter_context(tc.tile_pool(name="mmpsC", bufs=3, space="PSUM"))
    tpsC = None
    stgC = None
    if not USE_DMA_TRANSPOSE:
        tpsC = sC.enter_context(tc.tile_pool(name="tpsC", bufs=2, space="PSUM"))
        stgC = sC.enter_context(tc.tile_pool(name="stgC", bufs=2))

    def transpose_tile_C(h_tile, t):
        dst = h2T_all.rearrange("p f (tt c) -> p f tt c", c=128)[:, :, t, :]
        if USE_DMA_TRANSPOSE:
            nc.scalar.dma_start_transpose(dst, h_tile)
        else:
            for fb in range(8):
                pst = tpsC.tile([128, 128], BF16, tag="tp")
                nc.tensor.transpose(pst[:], h_tile[:, fb * 128:(fb + 1) * 128], ident[:])
                nc.vector.tensor_copy(dst[:, fb, :], pst[:])

    h2_prev = None
    for t in range(NT):
        for cc in range(2):
            ps = mmpsC.tile([128, 512], F32, tag="pj")
            for hp in range(8):
                nc.tensor.matmul(
                    ps[:],
                    ctxT_all[:, hp * T + t * 128: hp * T + (t + 1) * 128],
                    wproj_sb[:, hp, cc * 512:(cc + 1) * 512],
                    start=(hp == 0), stop=False,
                )
            nc.tensor.matmul(ps[:], ones_row[:], bvpo[:, H + cc * 512:H + (cc + 1) * 512],
                             start=False, stop=True)
            nc.vector.tensor_add(
                x2_all[:, t, cc * 512:(cc + 1) * 512],
                ps[:], x_all[:, t, cc * 512:(cc + 1) * 512])
        if h2_prev is not None:
            transpose_tile_C(h2_prev[0][:], h2_prev[1])
        rs, nmr = layer_norm_stats(lnp2, x2_all[:, t, :])
        h2 = h2p.tile([128, H], BF16, tag="h2")
        nc.scalar.activation(h2[:], x2_all[:, t, :], AF.Identity, bias=nmr[:], scale=rs[:])
        h2_prev = (h2, t)
    transpose_tile_C(h2_prev[0][:], h2_prev[1])

    sC.close()

    # =====================================================================
    # Phase D+E fused: per g: wi matmuls + gelu -> h3T[g], then wo matmuls
    # for output columns 0:512 accumulate into 4 persistent psums.
    # Second pass re-reads h3T for output columns 512:1024.
    # Weight streams ride the gpsimd SWDGE queue (batched loads).
    # =====================================================================
    sD = ExitStack()
    h3p = sD.enter_context(tc.tile_pool(name="h3p", bufs=1))
    wip = sD.enter_context(tc.tile_pool(name="wip", bufs=3))
    wop = sD.enter_context(tc.tile_pool(name="wop", bufs=2))
    mmpsD = sD.enter_context(tc.tile_pool(name="mmpsD", bufs=3, space="PSUM"))
    wops = sD.enter_context(tc.tile_pool(name="wops", bufs=1, space="PSUM"))
    outp = sD.enter_context(tc.tile_pool(name="outp", bufs=2))

    h3T_all = h3p.tile([128, NG, T], BF16, name="h3T_all")

    WIB = 4   # wi groups per DMA
    WOB = 8   # wo groups per DMA

    wi_tiles = [None] * (NG // WIB)
    wo_tiles = {}

    def load_wi(b):
        wi = wip.tile([128, WIB, 8, 128], BF16, tag="wi", name=f"wi{b}")
        src = io["wi"][b * WIB:(b + 1) * WIB].rearrange("g p f c -> p g f c")
        nc.gpsimd.dma_start(wi[:], src)
        return wi

    def load_wo(cc, b):
        wo = wop.tile([128, WOB, 512], BF16, tag="wo", name=f"wo{cc}_{b}")
        src = io["wo"][:, cc * 512:(cc + 1) * 512].rearrange(
            "(bb g p) f -> bb p g f", g=WOB, p=128)[b]
        nc.gpsimd.dma_start(wo[:], src)
        return wo

    # prefetch schedule on the gpsimd queue (wi bufs=3, wo bufs=2 gate it)
    wi_tiles[0] = load_wi(0)
    wi_tiles[1] = load_wi(1)
    wo_tiles[(0, 0)] = load_wo(0, 0)
    wi_tiles[2] = load_wi(2)
    wo_tiles[(0, 1)] = load_wo(0, 1)

    psE = [wops.tile([128, 512], F32, tag=f"wo_ps{t}", name=f"wo_ps{t}") for t in range(NT)]
    for g in range(NG):
        b, gi = g // WIB, g % WIB
        if b + 3 < len(wi_tiles) and wi_tiles[b + 3] is None and gi == 0:
            wi_tiles[b + 3] = load_wi(b + 3)
        wob = g // WOB
        if gi == 1 and b % 2 == 1 and (0, b // 2 + 2) not in wo_tiles and b // 2 + 2 < 4:
            wo_tiles[(0, b // 2 + 2)] = load_wo(0, b // 2 + 2)
        wi = wi_tiles[b]
        ps = mmpsD.tile([128, T], F32, tag="wi_ps", name="wi_ps")
        for fb in range(8):
            nc.tensor.matmul(
                ps[:], wi[:, gi, fb, :], h2T_all[:, fb, :],
                start=(fb == 0), stop=(fb == 7),
            )
        nc.scalar.activation(h3T_all[:, g, :], ps[:],
                             AF.Gelu_apprx_tanh, bias=bqki[:, 16 + g:17 + g])
        wo = wo_tiles[(0, wob)]
        for t in range(NT):
            nc.tensor.matmul(
                psE[t][:],
                h3T_all[:, g, t * 128:(t + 1) * 128],
                wo[:, g % WOB, :],
                start=(g == 0), stop=False,
            )
    ostage = outp.tile([128, NT, 512], F32, tag="ostage", name="ostage0")
    for t in range(NT):
        nc.tensor.matmul(psE[t][:], ones_row[:], bvpo[:, 2 * H:2 * H + 512],
                         start=False, stop=True)
        nc.vector.tensor_add(ostage[:, t, :], psE[t][:], x2_all[:, t, 0:512])
    nc.sync.dma_start(
        out_d[:, 0:512].rearrange("(t p) f -> p t f", p=128), ostage[:])

    # second pass: output columns 512:1024
    wo_tiles[(1, 0)] = load_wo(1, 0)
    wo_tiles[(1, 1)] = load_wo(1, 1)
    psE2 = [wops.tile([128, 512], F32, tag=f"wo_ps{t}", name=f"wo2_ps{t}") for t in range(NT)]
    for g in range(NG):
        wob = g // WOB
        if g % WOB == 1 and (1, wob + 2) not in wo_tiles and wob + 2 < 4:
            wo_tiles[(1, wob + 2)] = load_wo(1, wob + 2)
        wo = wo_tiles[(1, wob)]
        for t in range(NT):
            nc.tensor.matmul(
                psE2[t][:],
                h3T_all[:, g, t * 128:(t + 1) * 128],
                wo[:, g % WOB, :],
                start=(g == 0), stop=False,
            )
    ostage2 = outp.tile([128, NT, 512], F32, tag="ostage", name="ostage1")
    for t in range(NT):
        nc.tensor.matmul(psE2[t][:], ones_row[:], bvpo[:, 2 * H + 512:3 * H],
                         start=False, stop=True)
        nc.vector.tensor_add(ostage2[:, t, :], psE2[t][:], x2_all[:, t, 512:1024])
    nc.sync.dma_start(
        out_d[:, 512:1024].rearrange("(t p) f -> p t f", p=128), ostage2[:])

    sD.close()
    s_outer.close()


def _build_program():
    nc = bass.Bass("TRN2", target_bir_lowering=False, debug=False, num_devices=NC)
    io = {}
    io["x"] = nc.dram_tensor("x", [T, H], F32, kind="ExternalInput").ap()
    io["wq"] = nc.dram_tensor("wq", [H, H], BF16, kind="ExternalInput").ap()
    io["wk"] = nc.dram_tensor("wk", [H, H], BF16, kind="ExternalInput").ap()
    io["wv"] = nc.dram_tensor("wv", [H, H], BF16, kind="ExternalInput").ap()
    io["wproj"] = nc.dram_tensor("wproj", [H, H], BF16, kind="ExternalInput").ap()
    io["wi"] = nc.dram_tensor("wi", [NG, 128, 8, 128], BF16, kind="ExternalInput").ap()
    io["wo"] = nc.dram_tensor("wo", [FFN, H], BF16, kind="ExternalInput").ap()
    io["bqki"] = nc.dram_tensor("bqki", [128, 48], F32, kind="ExternalInput").ap()
    io["bvpo"] = nc.dram_tensor("bvpo", [1, 3 * H], BF16, kind="ExternalInput").ap()
    io["ident"] = nc.dram_tensor("ident", [128, 128], BF16, kind="ExternalInput").ap()
    io["ones_row"] = nc.dram_tensor("ones_row", [1, 128], BF16, kind="ExternalInput").ap()
    io["out"] = nc.dram_tensor("out", [T, H], F32, kind="ExternalOutput").ap()
    with tile.TileContext(nc) as tc:
        _emit(tc, nc, io)
    _split_multiwait(nc)
    return nc


_PROGRAM = None
LAST_RESULTS = None


def kernel(x, ln1_scale, ln1_bias, qkv_w, qkv_b, proj_w, proj_b,
           ln2_scale, ln2_bias, wi_w, wi_b, wo_w, wo_b):
    global _PROGRAM, LAST_RESULTS
    x = np.asarray(x, np.float32)
    ln1_scale = np.asarray(ln1_scale, np.float32); ln1_bias = np.asarray(ln1_bias, np.float32)
    qkv_w = np.asarray(qkv_w, np.float32); qkv_b = np.asarray(qkv_b, np.float32)
    proj_w = np.asarray(proj_w, np.float32); proj_b = np.asarray(proj_b, np.float32)
    ln2_scale = np.asarray(ln2_scale, np.float32); ln2_bias = np.asarray(ln2_bias, np.float32)
    wi_w = np.asarray(wi_w, np.float32); wi_b = np.asarray(wi_b, np.float32)
    wo_w = np.asarray(wo_w, np.float32); wo_b = np.asarray(wo_b, np.float32)

    # fold LN affine params into the next matmul's weights/biases
    qkv_w_eff = ln1_scale[:, None] * qkv_w
    qkv_b_eff = qkv_b + ln1_bias @ qkv_w
    w3 = qkv_w_eff.reshape(H, NH, 3, DH)
    b3 = qkv_b_eff.reshape(NH, 3, DH)
    scale = 1.0 / np.sqrt(np.float32(DH))
    wq = (w3[:, :, 0, :] * scale).reshape(H, H)
    wk = w3[:, :, 1, :].reshape(H, H)
    wv = w3[:, :, 2, :].reshape(H, H)
    bq_v = (b3[:, 0, :] * scale).reshape(H)
    bk_v = b3[:, 1, :].reshape(H)
    bv_v = b3[:, 2, :].reshape(H)
    wi_eff = ln2_scale[:, None] * wi_w
    bi_v = wi_b + ln2_bias @ wi_w

    bqki = np.concatenate([
        bq_v.reshape(8, 128).T, bk_v.reshape(8, 128).T,
        bi_v.reshape(32, 128).T], axis=1).astype(np.float32)
    bvpo = np.concatenate([bv_v, proj_b, wo_b]).reshape(1, 3 * H)

    common = {
        "wq": wq.astype(NPBF16), "wk": wk.astype(NPBF16), "wv": wv.astype(NPBF16),
        "wproj": proj_w.astype(NPBF16),
        "wi": np.ascontiguousarray(
            wi_eff.astype(NPBF16).reshape(8, 128, 32, 128).transpose(2, 1, 0, 3)),
        "wo": wo_w.astype(NPBF16),
        "bqki": np.ascontiguousarray(bqki),
        "bvpo": bvpo.astype(NPBF16),
        "ident": np.eye(128, dtype=NPBF16),
        "ones_row": np.ones((1, 128), NPBF16),
    }
    x_flat = x.reshape(B * S, H)
    in_maps = []
    for c in range(NC):
        m = dict(common)
        m["x"] = np.ascontiguousarray(x_flat[c * T:(c + 1) * T, :])
        in_maps.append(m)

    if _PROGRAM is None:
        _PROGRAM = _build_program()
    r = run_bass_kernel_spmd(_PROGRAM, in_maps, list(range(NC)))
    LAST_RESULTS = r
    out = np.concatenate([r.results[c]["out"] for c in range(NC)], axis=0)
    return out.reshape(B, S, H).astype(np.float32)


# revision 12
# speedup vs baseline: 1.3270x; 1.0956x over previous
"""Trainium2 Bass kernel for a pre-LN transformer block (B=2,S=2048,H=1024,NH=16,FFN=4096).

Sharding: 8 cores, 512 tokens/core (4 cores per batch element). K/V are
exchanged within each batch group via four 4-rank AllGathers (K and V each in
two head-halves) so attention on early head-pairs overlaps the later gathers.
All matmuls run in bf16 on the PE array with fp32 PSUM accumulation; LayerNorm
statistics, residuals and the final output stay fp32.

Self-contained: hardcodes shapes; builds the Bass program once and runs it via
run_bass_kernel_spmd on cores 0-7.
"""

import sys

for _p in ("/root/.axon_site/_ro/trn_rl_repo", "/opt/trn_rl_repo"):
    if _p not in sys.path:
        sys.path.append(_p)

import numpy as np
import ml_dtypes

# If BASS_TRACE is set but the axon NTFF hook module is missing, the trace
# path would crash on import; pre-register a no-op hook shim so tracing
# degrades gracefully instead.
try:
    import antenv.axon_hooks  # noqa: F401
except ImportError:
    import types as _types
    _m = _types.ModuleType("antenv.axon_hooks")
    _m._hook = None
    _m.get_axon_ntff_profile_hook = lambda: _m._hook
    _m.set_axon_ntff_profile_hook = lambda h: setattr(_m, "_hook", h)
    sys.modules["antenv.axon_hooks"] = _m

import bass_rust
import concourse.bass as bass
import concourse.mybir as mybir
import concourse.tile as tile
from concourse.bass_utils import run_bass_kernel_spmd

BF16 = mybir.dt.bfloat16
F32 = mybir.dt.float32
F8 = mybir.dt.float8e4
LN8 = 2.0794415416798357  # exp scale: probs stored as 8*exp(s), cancels in normalize
AF = mybir.ActivationFunctionType
NPBF16 = np.dtype(ml_dtypes.bfloat16)

B, S, H, NH, DH, FFN = 2, 2048, 1024, 16, 64, 4096
NC = 8                      # cores
T = 512                     # tokens per core
NT = T // 128               # token tiles per core (4)
GROUPS = [[0, 1, 2, 3], [4, 5, 6, 7]]
G = 4                       # cores per batch group
NKT = 16                    # key tiles per batch (4 ranks x 4)
NHP = NH // 2               # head pairs (8)
EPS = 1e-3
VW = DH + 1                 # 65: V columns + ones column per head
NG = FFN // 128             # 32 ffn row-groups

USE_DMA_TRANSPOSE = False   # xbar DMA transpose serializes ~6us/tile; PE wins

# ---------------------------------------------------------------------------
# Workaround: this walrus build rejects >1 inline sync-wait per instruction.
# After Tile scheduling, move excess waits onto single-wait NoOp carriers
# inserted immediately before the over-limit instruction (same engine, same
# block, so per-engine program order and wait semantics are preserved).
# ---------------------------------------------------------------------------
def _split_multiwait(nc, limit=1):
    n_new = 0
    for f in nc.m.functions:
        for blk in f.blocks:
            insts = blk.instructions
            out = []
            for ins in insts:
                si = getattr(ins, "sync_info", None)
                waits = list(si.on_wait) if si is not None else []
                if len(waits) > limit:
                    for i, w in enumerate(waits[:-limit]):
                        nop = mybir.InstNoOp(
                            name=f"{ins.name}_w{i}",
                            sync_info=mybir.SyncInfo(on_wait=[w], on_update=[]),
                            bass_nofuse=True,
                            engine=ins.engine,
                        )
                        out.append(nop)
                        n_new += 1
                    ins.sync_info = mybir.SyncInfo(
                        on_wait=waits[-limit:], on_update=list(si.on_update)
                    )
                out.append(ins)
            if len(out) != len(insts):
                blk.instructions = out
    return n_new


def _emit(tc, nc, io):
    """Emit the per-core program. io: dict of DRAM APs."""
    from contextlib import ExitStack

    x_d = io["x"]
    out_d = io["out"]

    s_outer = ExitStack()

    constp = s_outer.enter_context(tc.tile_pool(name="constp", bufs=1))
    dramp = s_outer.enter_context(tc.tile_pool(name="dramp", bufs=1, space="DRAM"))

    # ---- phase-A bulk loads, one descriptor-dense DMA each (SP queue) ----
    persp = s_outer.enter_context(tc.tile_pool(name="persp", bufs=1))
    x_all = persp.tile([128, NT, H], F32, name="x_all")
    nc.sync.dma_start(x_all[:], x_d.rearrange("(t p) h -> p t h", p=128))

    sA = ExitStack()
    wpool = sA.enter_context(tc.tile_pool(name="wpool", bufs=3))
    wk_sb = wpool.tile([128, 8, H], BF16, tag="w3", name="wk_sb")
    nc.sync.dma_start(wk_sb[:], io["wk"].rearrange("(f p) h -> p f h", p=128))

    # constants / biases
    ident = constp.tile([128, 128], BF16)
    nc.sync.dma_start(ident[:], io["ident"][:])
    ones_row = constp.tile([1, 128], BF16)
    nc.sync.dma_start(ones_row[:], io["ones_row"][:])
    # bqki: cols 0:8 = bq, 8:16 = bk, 16:48 = bi (all [128, n])
    bqki = constp.tile([128, 48], F32)
    nc.sync.dma_start(bqki[:], io["bqki"][:])
    # bvpo: [1, 3H] bf16: bv | bproj | bo
    bvpo = constp.tile([1, 3 * H], BF16)
    nc.sync.dma_start(bvpo[:], io["bvpo"][:])
    eps_t = constp.tile([128, 1], F32)
    nc.gpsimd.memset(eps_t[:], float(EPS))
    ln8_t = constp.tile([128, 1], F32)
    nc.gpsimd.memset(ln8_t[:], float(LN8))

    wq_sb = wpool.tile([128, 8, H], BF16, tag="w3", name="wq_sb")
    nc.sync.dma_start(wq_sb[:], io["wq"].rearrange("(f p) h -> p f h", p=128))
    wv_sb = wpool.tile([128, 8, H], BF16, tag="w3", name="wv_sb")
    nc.sync.dma_start(wv_sb[:], io["wv"].rearrange("(f p) h -> p f h", p=128))

    # persistent activations
    x2_all = persp.tile([128, NT, H], F32, name="x2_all")
    qT_all = persp.tile([128, 8, T], F8, name="qT_all")
    ctxT_all = persp.tile([128, 8 * T], BF16, name="ctxT_all")
    h2T_all = persp.tile([128, 8, T], BF16, name="h2T_all")
    wproj_sb = persp.tile([128, 8, H], BF16, name="wproj_sb")

    # combined K+V collective buffers per head-half, fp8: rows 0:512 hold
    # kT (4 ct tiles), rows 512:1024 hold v (4 token tiles), both [*, 512]
    cc_kv_in_a = dramp.tile([1024, 512], F8)
    cc_kv_in_b = dramp.tile([1024, 512], F8)
    cc_kv_out_a = dramp.tile([G * 1024, 512], F8)
    cc_kv_out_b = dramp.tile([G * 1024, 512], F8)

    def layer_norm_stats(pool, x_slice):
        """x_slice [128,H] f32 -> (rs [128,1], nmr [128,1]) in SBUF."""
        stats = pool.tile([128, 2, 6], F32, tag="ln_st")
        mv = pool.tile([128, 2], F32, tag="ln_mv")
        std = pool.tile([128, 1], F32, tag="ln_std")
        rs = pool.tile([128, 1], F32, tag="ln_rs")
        nmr = pool.tile([128, 1], F32, tag="ln_nmr")
        xc = x_slice.rearrange("p (n c) -> p n c", c=512)
        nc.vector.bn_stats(out=stats[:, 0, :], in_=xc[:, 0, :])
        nc.vector.bn_stats(out=stats[:, 1, :], in_=xc[:, 1, :])
        nc.vector.bn_aggr(out=mv[:], in_=stats[:])
        nc.scalar.activation(std[:], mv[:, 1:2], AF.Sqrt, bias=eps_t[:])
        nc.vector.reciprocal(rs[:], std[:])
        nc.vector.tensor_mul(nmr[:], mv[:, 0:1], rs[:])
        nc.vector.tensor_scalar_mul(nmr[:], nmr[:], -1.0)
        return rs, nmr

    # =====================================================================
    # Phase A: load x, LN1 -> h1, h1T (xbar transpose), kT, v, qT.
    # K/V AllGathers (by head-half) dispatched as soon as inputs land.
    # =====================================================================
    lnp = sA.enter_context(tc.tile_pool(name="lnp", bufs=2))
    h1p = sA.enter_context(tc.tile_pool(name="h1p", bufs=2))
    h1Tp = sA.enter_context(tc.tile_pool(name="h1Tp", bufs=1))
    ktp = sA.enter_context(tc.tile_pool(name="ktp", bufs=1))
    vlp = sA.enter_context(tc.tile_pool(name="vlp", bufs=2))
    mmpsA = sA.enter_context(tc.tile_pool(name="mmpsA", bufs=3, space="PSUM"))
    tpsA = None
    stgA = None
    if not USE_DMA_TRANSPOSE:
        tpsA = sA.enter_context(tc.tile_pool(name="tpsA", bufs=2, space="PSUM"))
        stgA = sA.enter_context(tc.tile_pool(name="stgA", bufs=2))

    h1T_all = h1Tp.tile([128, 8, T], BF16, name="h1T_all")
    ktA_sb = ktp.tile([128, 4, T], F8, name="ktA_sb")
    ktB_sb = ktp.tile([128, 4, T], F8, name="ktB_sb")

    def transpose_tile(dstT_all, h_tile, t):
        """h_tile [128, H] bf16 -> dstT_all[:, fb, t*128:(t+1)*128] for all fb."""
        dst = dstT_all.rearrange("p f (tt c) -> p f tt c", c=128)[:, :, t, :]
        if USE_DMA_TRANSPOSE:
            nc.scalar.dma_start_transpose(dst, h_tile)
        else:
            for fb in range(8):
                ps = tpsA.tile([128, 128], BF16, tag="tp")
                nc.tensor.transpose(ps[:], h_tile[:, fb * 128:(fb + 1) * 128], ident[:])
                nc.vector.tensor_copy(dst[:, fb, :], ps[:])

    for t in range(NT):
        rs, nmr = layer_norm_stats(lnp, x_all[:, t, :])
        h1 = h1p.tile([128, H], BF16, tag="h1")
        nc.scalar.activation(h1[:], x_all[:, t, :], AF.Identity, bias=nmr[:], scale=rs[:])
        transpose_tile(h1T_all, h1[:], t)

    def ag(cc_in, cc_out):
        nc.gpsimd.collective_compute(
            "AllGather", mybir.AluOpType.bypass, replica_groups=GROUPS,
            ins=[cc_in.opt()], outs=[cc_out.opt()])

    # kT feature-major: [128 feats(head pair), T local keys] per ct
    def emit_k_quarter(cts, dst):
        for ct in cts:
            ps = mmpsA.tile([128, T], F32, tag="mmA")
            for fb in range(8):
                nc.tensor.matmul(
                    ps[:],
                    wk_sb[:, fb, ct * 128:(ct + 1) * 128],
                    h1T_all[:, fb, :],
                    start=(fb == 0), stop=(fb == 7),
                )
            nc.vector.tensor_scalar_add(dst[:, ct % 4, :], ps[:], bqki[:, 8 + ct:9 + ct])

    # v token-major, feature half cc: [128 tok, 512]
    def emit_v_half(cc, vloc, cc_kv_in):
        for t in range(NT):
            ps = mmpsA.tile([128, 512], F32, tag="mmA")
            for fb in range(8):
                nc.tensor.matmul(
                    ps[:],
                    h1T_all[:, fb, t * 128:(t + 1) * 128],
                    wv_sb[:, fb, cc * 512:(cc + 1) * 512],
                    start=(fb == 0), stop=False,
                )
            nc.tensor.matmul(ps[:], ones_row[:], bvpo[:, cc * 512:(cc + 1) * 512],
                             start=False, stop=True)
            nc.scalar.copy(vloc[:, t, :], ps[:])
        nc.sync.dma_start(
            cc_kv_in[512:1024].rearrange("(t p) f -> p t f", p=128), vloc[:])

    def emit_q_quarter(cts):
        for ct in cts:
            ps = mmpsA.tile([128, T], F32, tag="mmA")
            for fb in range(8):
                nc.tensor.matmul(
                    ps[:],
                    wq_sb[:, fb, ct * 128:(ct + 1) * 128],
                    h1T_all[:, fb, :],
                    start=(fb == 0), stop=(fb == 7),
                )
            nc.vector.tensor_scalar_add(qT_all[:, ct, :], ps[:], bqki[:, ct:ct + 1])

    # interleave projections with the two combined K+V gathers (head-half
    # A then B) so the serial CC engine runs them back-to-back.
    emit_k_quarter(range(0, 4), ktA_sb)
    nc.sync.dma_start(
        cc_kv_in_a[0:512].rearrange("(c p) k -> p c k", p=128), ktA_sb[:])
    vloc_a = vlp.tile([128, NT, 512], F8, tag="vloc", name="vloc_a")
    emit_v_half(0, vloc_a, cc_kv_in_a)
    ag(cc_kv_in_a, cc_kv_out_a)
    emit_k_quarter(range(4, 8), ktB_sb)
    nc.sync.dma_start(
        cc_kv_in_b[0:512].rearrange("(c p) k -> p c k", p=128), ktB_sb[:])
    emit_q_quarter(range(0, 4))
    vloc_b = vlp.tile([128, NT, 512], F8, tag="vloc", name="vloc_b")
    emit_v_half(1, vloc_b, cc_kv_in_b)
    ag(cc_kv_in_b, cc_kv_out_b)
    emit_q_quarter(range(4, 8))

    nc.sync.dma_start(wproj_sb[:], io["wproj"].rearrange("(f p) h -> p f h", p=128))

    sA.close()

    # =====================================================================
    # Phase B: attention. scores^T per key-tile (row-packed head pairs),
    # exp on ACT, ctx^T via V'=[V|ones] (M=65), fast normalize at hp end.
    # =====================================================================
    sB = ExitStack()
    ktpool = sB.enter_context(tc.tile_pool(name="ktpool", bufs=5))
    vstg = sB.enter_context(tc.tile_pool(name="vstg", bufs=3))
    spool = sB.enter_context(tc.tile_pool(name="spool", bufs=2, space="PSUM"))
    cpool = sB.enter_context(tc.tile_pool(name="cpool", bufs=2, space="PSUM"))
    ppool = sB.enter_context(tc.tile_pool(name="ppool", bufs=8))
    rpool = sB.enter_context(tc.tile_pool(name="rpool", bufs=3))
    vsbp = sB.enter_context(tc.tile_pool(name="vsbp", bufs=1))
    vsb_a = vsbp.tile([128, NKT, 8, VW], F8, name="vsb_a")
    vsb_b = vsbp.tile([128, NKT, 8, VW], F8, name="vsb_b")

    # ones columns for all key tiles (DVE; no data deps)
    nc.vector.memset(vsb_a[:, :, :, DH:VW], 1.0)
    nc.vector.memset(vsb_b[:, :, :, DH:VW], 1.0)

    def load_kt(hp):
        """kT for head pair hp: [128 feats, 4 rank-blocks, 512 keys]."""
        kt = ktpool.tile([128, G, T], F8, tag="kt", name=f"kt{hp}")
        cko, hpo = (cc_kv_out_a, hp) if hp < 4 else (cc_kv_out_b, hp - 4)
        src = cko.rearrange("(g u p) k -> u p g k", g=G, u=8, p=128)[hpo]
        nc.sync.dma_start(kt[:], src)
        return kt

    def load_v_half(vsb, cc_kv_out):
        """Interleave gathered V [tokens, 512] into vsb [128, kt, head, VW]."""
        for g in range(G):
            vplain = vstg.tile([128, NT, 512], F8, tag="vplain", name=f"vp{g}")
            src = cc_kv_out.rearrange("(g u p) f -> g p u f", g=G, u=8, p=128)[g][:, 4:8, :]
            nc.sync.dma_start(vplain[:], src)
            for t in range(NT):
                kt = g * 4 + t
                nc.vector.tensor_copy(
                    vsb[:, kt, :, 0:DH],
                    vplain[:, t, :].rearrange("p (h d) -> p h d", d=DH),
                )

    def attend_hp(hp, kt_hp, vsb, pending_tail):
        """Returns a closure finishing this hp's softmax-normalize; the PE
        part of the previous hp's tail is emitted mid-loop (at kt==3) so
        the PE queue never stalls waiting on the DVE reciprocal chain."""
        hh = (hp % 4) * 2  # head-within-half base index
        cps0 = cpool.tile([128, T], F32, tag="ctx0")
        cps1 = cpool.tile([128, T], F32, tag="ctx1")

        def emit_ctx(kt, pb, first, last):
            for h, cps in enumerate((cps0, cps1)):
                nc.tensor.matmul(
                    cps[0:VW, :],
                    vsb[:, kt, hh + h, :],
                    pb[:, h * T:(h + 1) * T],
                    start=first, stop=last,
                )

        # software-pipelined: scores(kt) | ctx(kt-1) | exp(kt)
        prev = None
        for kt in range(NKT):
            g, j = kt // 4, kt % 4
            if kt == 3 and pending_tail is not None:
                pending_tail()
                pending_tail = None
            ps = spool.tile([128, 1024], F32, tag="ps", name="ps")
            nc.tensor.matmul(
                ps[:, 0:T],
                kt_hp[0:64, g, j * 128:(j + 1) * 128],
                qT_all[0:64, hp, :],
                start=True, stop=True, tile_position=(0, 0),
            )
            nc.tensor.matmul(
                ps[:, T:1024],
                kt_hp[64:128, g, j * 128:(j + 1) * 128],
                qT_all[64:128, hp, :],
                start=True, stop=True, tile_position=(64, 0),
            )
            if prev is not None:
                emit_ctx(prev[0], prev[1], prev[2], False)
            pb = ppool.tile([128, 1024], F8, tag="pb", name="pb")
            nc.scalar.activation(pb[:], ps[:], AF.Exp, bias=ln8_t[:])
            prev = (kt, pb, kt == 0)
        emit_ctx(prev[0], prev[1], prev[2], True)

        # normalize: 1/sumexp (row 64) on DVE now; broadcast via a K=1
        # matmul into the same tile's spare rows 64:128 (no extra PSUM
        # bank) deferred into the next hp's score stream.
        tails = []
        for h, cps in enumerate((cps0, cps1)):
            rc = rpool.tile([1, T], F32, tag=f"rc{h}")
            rcb = rpool.tile([1, T], BF16, tag=f"rcb{h}")
            nc.vector.reciprocal(rc[:], cps[DH:VW, :])
            nc.vector.tensor_copy(rcb[:], rc[:])
            tails.append((h, cps, rcb))

        def tail():
            for h, cps, rcb in tails:
                rbs = rpool.tile([64, T], F32, tag=f"rbs{h}")
                nc.tensor.matmul(cps[64:128, :], ones_row[:, 0:64], rcb[:],
                                 start=True, stop=True, tile_position=(0, 64))
                nc.vector.tensor_copy(rbs[:], cps[64:128, :])
                nc.vector.tensor_mul(
                    ctxT_all[h * 64:(h + 1) * 64, hp * T:(hp + 1) * T],
                    cps[0:DH, :], rbs[:])
        return tail

    # half A
    kt_tiles = {}
    kt_tiles[0] = load_kt(0)
    kt_tiles[1] = load_kt(1)
    load_v_half(vsb_a, cc_kv_out_a)
    kt_tiles[2] = load_kt(2)
    kt_tiles[3] = load_kt(3)
    tail = None
    for hp in range(4):
        tail = attend_hp(hp, kt_tiles[hp], vsb_a, tail)
    # half B
    kt_tiles[4] = load_kt(4)
    load_v_half(vsb_b, cc_kv_out_b)
    for hp in range(4, NHP):
        if hp + 1 < NHP:
            kt_tiles[hp + 1] = load_kt(hp + 1)
        tail = attend_hp(hp, kt_tiles[hp], vsb_b, tail)
    tail()  # last hp's normalize, right before proj consumes ctxT

    sB.close()

    # =====================================================================
    # Phase C: proj (token-major) + residual -> x2, LN2 -> h2T
    # =====================================================================
    sC = ExitStack()
    lnp2 = sC.enter_context(tc.tile_pool(name="lnp2", bufs=2))
    h2p = sC.enter_context(tc.tile_pool(name="h2p", bufs=2))
    mmpsC = sC.enter_context(tc.tile_pool(name="mmpsC", bufs=3, space="PSUM"))
    tpsC = None
    stgC = None
    if not USE_DMA_TRANSPOSE:
        tpsC = sC.enter_context(tc.tile_pool(name="tpsC", bufs=2, space="PSUM"))
        stgC = sC.enter_context(tc.tile_pool(name="stgC", bufs=2))

    def transpose_tile_C(h_tile, t):
        dst = h2T_all.rearrange("p f (tt c) -> p f tt c", c=128)[:, :, t, :]
        if USE_DMA_TRANSPOSE:
            nc.scalar.dma_start_transpose(dst, h_tile)
        else:
            for fb in range(8):
                pst = tpsC.tile([128, 128], BF16, tag="tp")
                nc.tensor.transpose(pst[:], h_tile[:, fb * 128:(fb + 1) * 128], ident[:])
                nc.vector.tensor_copy(dst[:, fb, :], pst[:])

    h2_prev = None
    for t in range(NT):
        for cc in range(2):
            ps = mmpsC.tile([128, 512], F32, tag="pj")
            for hp in range(8):
                nc.tensor.matmul(
                    ps[:],
                    ctxT_all[:, hp * T + t * 128: hp * T + (t + 1) * 128],
                    wproj_sb[:, hp, cc * 512:(cc + 1) * 512],
                    start=(hp == 0), stop=False,
                )
            nc.tensor.matmul(ps[:], ones_row[:], bvpo[:, H + cc * 512:H + (cc + 1) * 512],
                             start=False, stop=True)
            nc.vector.tensor_add(
                x2_all[:, t, cc * 512:(cc + 1) * 512],
                ps[:], x_all[:, t, cc * 512:(cc + 1) * 512])
        if h2_prev is not None:
            transpose_tile_C(h2_prev[0][:], h2_prev[1])
        rs, nmr = layer_norm_stats(lnp2, x2_all[:, t, :])
        h2 = h2p.tile([128, H], BF16, tag="h2")
        nc.scalar.activation(h2[:], x2_all[:, t, :], AF.Identity, bias=nmr[:], scale=rs[:])
        h2_prev = (h2, t)
    transpose_tile_C(h2_prev[0][:], h2_prev[1])

    sC.close()

    # =====================================================================
    # Phase D+E fused: per g: wi matmuls + gelu -> h3T[g], then wo matmuls
    # for output columns 0:512 accumulate into 4 persistent psums.
    # Second pass re-reads h3T for output columns 512:1024.
    # Weight streams ride the gpsimd SWDGE queue (batched loads).
    # =====================================================================
    sD = ExitStack()
    h3p = sD.enter_context(tc.tile_pool(name="h3p", bufs=1))
    wip = sD.enter_context(tc.tile_pool(name="wip", bufs=3))
    wop = sD.enter_context(tc.tile_pool(name="wop", bufs=2))
    mmpsD = sD.enter_context(tc.tile_pool(name="mmpsD", bufs=3, space="PSUM"))
    wops = sD.enter_context(tc.tile_pool(name="wops", bufs=1, space="PSUM"))
    outp = sD.enter_context(tc.tile_pool(name="outp", bufs=2))

    h3T_all = h3p.tile([128, NG, T], BF16, name="h3T_all")

    WIB = 4   # wi groups per DMA
    WOB = 8   # wo groups per DMA

    wi_tiles = [None] * (NG // WIB)
    wo_tiles = {}

    def load_wi(b):
        wi = wip.tile([128, WIB, 8, 128], BF16, tag="wi", name=f"wi{b}")
        src = io["wi"][b * WIB:(b + 1) * WIB].rearrange("g p f c -> p g f c")
        nc.gpsimd.dma_start(wi[:], src)
        return wi

    def load_wo(cc, b):
        wo = wop.tile([128, WOB, 512], BF16, tag="wo", name=f"wo{cc}_{b}")
        src = io["wo"][:, cc * 512:(cc + 1) * 512].rearrange(
            "(bb g p) f -> bb p g f", g=WOB, p=128)[b]
        nc.gpsimd.dma_start(wo[:], src)
        return wo

    # prefetch schedule on the gpsimd queue (wi bufs=3, wo bufs=2 gate it)
    wi_tiles[0] = load_wi(0)
    wi_tiles[1] = load_wi(1)
    wo_tiles[(0, 0)] = load_wo(0, 0)
    wi_tiles[2] = load_wi(2)
    wo_tiles[(0, 1)] = load_wo(0, 1)

    psE = [wops.tile([128, 512], F32, tag=f"wo_ps{t}", name=f"wo_ps{t}") for t in range(NT)]
    for g in range(NG):
        b, gi = g // WIB, g % WIB
        if b + 3 < len(wi_tiles) and wi_tiles[b + 3] is None and gi == 0:
            wi_tiles[b + 3] = load_wi(b + 3)
        wob = g // WOB
        if gi == 1 and b % 2 == 1 and (0, b // 2 + 2) not in wo_tiles and b // 2 + 2 < 4:
            wo_tiles[(0, b // 2 + 2)] = load_wo(0, b // 2 + 2)
        wi = wi_tiles[b]
        ps = mmpsD.tile([128, T], F32, tag="wi_ps", name="wi_ps")
        for fb in range(8):
            nc.tensor.matmul(
                ps[:], wi[:, gi, fb, :], h2T_all[:, fb, :],
                start=(fb == 0), stop=(fb == 7),
            )
        nc.scalar.activation(h3T_all[:, g, :], ps[:],
                             AF.Gelu_apprx_tanh, bias=bqki[:, 16 + g:17 + g])
        wo = wo_tiles[(0, wob)]
        for t in range(NT):
            nc.tensor.matmul(
                psE[t][:],
                h3T_all[:, g, t * 128:(t + 1) * 128],
                wo[:, g % WOB, :],
                start=(g == 0), stop=False,
            )
    ostage = outp.tile([128, NT, 512], F32, tag="ostage", name="ostage0")
    for t in range(NT):
        nc.tensor.matmul(psE[t][:], ones_row[:], bvpo[:, 2 * H:2 * H + 512],
                         start=False, stop=True)
        nc.vector.tensor_add(ostage[:, t, :], psE[t][:], x2_all[:, t, 0:512])
    nc.sync.dma_start(
        out_d[:, 0:512].rearrange("(t p) f -> p t f", p=128), ostage[:])

    # second pass: output columns 512:1024
    wo_tiles[(1, 0)] = load_wo(1, 0)
    wo_tiles[(1, 1)] = load_wo(1, 1)
    psE2 = [wops.tile([128, 512], F32, tag=f"wo_ps{t}", name=f"wo2_ps{t}") for t in range(NT)]
    for g in range(NG):
        wob = g // WOB
        if g % WOB == 1 and (1, wob + 2) not in wo_tiles and wob + 2 < 4:
            wo_tiles[(1, wob + 2)] = load_wo(1, wob + 2)
        wo = wo_tiles[(1, wob)]
        for t in range(NT):
            nc.tensor.matmul(
                psE2[t][:],
                h3T_all[:, g, t * 128:(t + 1) * 128],
                wo[:, g % WOB, :],
                start=(g == 0), stop=False,
            )
    ostage2 = outp.tile([128, NT, 512], F32, tag="ostage", name="ostage1")
    for t in range(NT):
        nc.tensor.matmul(psE2[t][:], ones_row[:], bvpo[:, 2 * H + 512:3 * H],
                         start=False, stop=True)
        nc.vector.tensor_add(ostage2[:, t, :], psE2[t][:], x2_all[:, t, 512:1024])
    nc.sync.dma_start(
        out_d[:, 512:1024].rearrange("(t p) f -> p t f", p=128), ostage2[:])

    sD.close()
    s_outer.close()


def _build_program():
    nc = bass.Bass("TRN2", target_bir_lowering=False, debug=False, num_devices=NC)
    io = {}
    io["x"] = nc.dram_tensor("x", [T, H], F32, kind="ExternalInput").ap()
    io["wq"] = nc.dram_tensor("wq", [H, H], BF16, kind="ExternalInput").ap()
    io["wk"] = nc.dram_tensor("wk", [H, H], BF16, kind="ExternalInput").ap()
    io["wv"] = nc.dram_tensor("wv", [H, H], BF16, kind="ExternalInput").ap()
    io["wproj"] = nc.dram_tensor("wproj", [H, H], BF16, kind="ExternalInput").ap()
    io["wi"] = nc.dram_tensor("wi", [NG, 128, 8, 128], BF16, kind="ExternalInput").ap()
    io["wo"] = nc.dram_tensor("wo", [FFN, H], BF16, kind="ExternalInput").ap()
    io["bqki"] = nc.dram_tensor("bqki", [128, 48], F32, kind="ExternalInput").ap()
    io["bvpo"] = nc.dram_tensor("bvpo", [1, 3 * H], BF16, kind="ExternalInput").ap()
    io["ident"] = nc.dram_tensor("ident", [128, 128], BF16, kind="ExternalInput").ap()
    io["ones_row"] = nc.dram_tensor("ones_row", [1, 128], BF16, kind="ExternalInput").ap()
    io["out"] = nc.dram_tensor("out", [T, H], F32, kind="ExternalOutput").ap()
    with tile.TileContext(nc) as tc:
        _emit(tc, nc, io)
    _split_multiwait(nc)
    return nc


_PROGRAM = None
LAST_RESULTS = None


def kernel(x, ln1_scale, ln1_bias, qkv_w, qkv_b, proj_w, proj_b,
           ln2_scale, ln2_bias, wi_w, wi_b, wo_w, wo_b):
    global _PROGRAM, LAST_RESULTS
    x = np.asarray(x, np.float32)
    ln1_scale = np.asarray(ln1_scale, np.float32); ln1_bias = np.asarray(ln1_bias, np.float32)
    qkv_w = np.asarray(qkv_w, np.float32); qkv_b = np.asarray(qkv_b, np.float32)
    proj_w = np.asarray(proj_w, np.float32); proj_b = np.asarray(proj_b, np.float32)
    ln2_scale = np.asarray(ln2_scale, np.float32); ln2_bias = np.asarray(ln2_bias, np.float32)
    wi_w = np.asarray(wi_w, np.float32); wi_b = np.asarray(wi_b, np.float32)
    wo_w = np.asarray(wo_w, np.float32); wo_b = np.asarray(wo_b, np.float32)

    # fold LN affine params into the next matmul's weights/biases
    qkv_w_eff = ln1_scale[:, None] * qkv_w
    qkv_b_eff = qkv_b + ln1_bias @ qkv_w
    w3 = qkv_w_eff.reshape(H, NH, 3, DH)
    b3 = qkv_b_eff.reshape(NH, 3, DH)
    scale = 1.0 / np.sqrt(np.float32(DH))
    wq = (w3[:, :, 0, :] * scale).reshape(H, H)
    wk = w3[:, :, 1, :].reshape(H, H)
    wv = w3[:, :, 2, :].reshape(H, H)
    bq_v = (b3[:, 0, :] * scale).reshape(H)
    bk_v = b3[:, 1, :].reshape(H)
    bv_v = b3[:, 2, :].reshape(H)
    wi_eff = ln2_scale[:, None] * wi_w
    bi_v = wi_b + ln2_bias @ wi_w

    bqki = np.concatenate([
        bq_v.reshape(8, 128).T, bk_v.reshape(8, 128).T,
        bi_v.reshape(32, 128).T], axis=1).astype(np.float32)
    bvpo = np.concatenate([bv_v, proj_b, wo_b]).reshape(1, 3 * H)

    common = {
        "wq": wq.astype(NPBF16), "wk": wk.astype(NPBF16), "wv": wv.astype(NPBF16),
        "wproj": proj_w.astype(NPBF16),
        "wi": np.ascontiguousarray(
            wi_eff.astype(NPBF16).reshape(8, 128, 32, 128).transpose(2, 1, 0, 3)),
        "wo": wo_w.astype(NPBF16),
        "bqki": np.ascontiguousarray(bqki),
        "bvpo": bvpo.astype(NPBF16),
        "ident": np.eye(128, dtype=NPBF16),
        "ones_row": np.ones((1, 128), NPBF16),
    }
    x_flat = x.reshape(B * S, H)
    in_maps = []
    for c in range(NC):
        m = dict(common)
        m["x"] = np.ascontiguousarray(x_flat[c * T:(c + 1) * T, :])
        in_maps.append(m)

    if _PROGRAM is None:
        _PROGRAM = _build_program()
    r = run_bass_kernel_spmd(_PROGRAM, in_maps, list(range(NC)))
    LAST_RESULTS = r
    out = np.concatenate([r.results[c]["out"] for c in range(NC)], axis=0)
    return out.reshape(B, S, H).astype(np.float32)


# revision 15
# speedup vs baseline: 1.3976x; 1.0532x over previous
"""Trainium2 Bass kernel for a pre-LN transformer block (B=2,S=2048,H=1024,NH=16,FFN=4096).

Sharding: 8 cores, 512 tokens/core (4 cores per batch element). K/V are
exchanged within each batch group via four 4-rank AllGathers (K and V each in
two head-halves) so attention on early head-pairs overlaps the later gathers.
All matmuls run in bf16 on the PE array with fp32 PSUM accumulation; LayerNorm
statistics, residuals and the final output stay fp32.

Self-contained: hardcodes shapes; builds the Bass program once and runs it via
run_bass_kernel_spmd on cores 0-7.
"""

import sys

for _p in ("/root/.axon_site/_ro/trn_rl_repo", "/opt/trn_rl_repo"):
    if _p not in sys.path:
        sys.path.append(_p)

import numpy as np
import ml_dtypes

# If BASS_TRACE is set but the axon NTFF hook module is missing, the trace
# path would crash on import; pre-register a no-op hook shim so tracing
# degrades gracefully instead.
try:
    import antenv.axon_hooks  # noqa: F401
except ImportError:
    import types as _types
    _m = _types.ModuleType("antenv.axon_hooks")
    _m._hook = None
    _m.get_axon_ntff_profile_hook = lambda: _m._hook
    _m.set_axon_ntff_profile_hook = lambda h: setattr(_m, "_hook", h)
    sys.modules["antenv.axon_hooks"] = _m

import bass_rust
import concourse.bass as bass
import concourse.mybir as mybir
import concourse.tile as tile
from concourse.bass_utils import run_bass_kernel_spmd

BF16 = mybir.dt.bfloat16
F32 = mybir.dt.float32
F8 = mybir.dt.float8e4
LN8 = 2.0794415416798357  # exp scale: probs stored as 8*exp(s), cancels in normalize
AF = mybir.ActivationFunctionType
NPBF16 = np.dtype(ml_dtypes.bfloat16)

B, S, H, NH, DH, FFN = 2, 2048, 1024, 16, 64, 4096
NC = 8                      # cores
T = 512                     # tokens per core
NT = T // 128               # token tiles per core (4)
GROUPS = [[0, 1, 2, 3], [4, 5, 6, 7]]
G = 4                       # cores per batch group
NKT = 16                    # key tiles per batch (4 ranks x 4)
NHP = NH // 2               # head pairs (8)
EPS = 1e-3
VW = DH + 1                 # 65: V columns + ones column per head
NG = FFN // 128             # 32 ffn row-groups

USE_DMA_TRANSPOSE = False   # xbar DMA transpose serializes ~6us/tile; PE wins

# ---------------------------------------------------------------------------
# Workaround: this walrus build rejects >1 inline sync-wait per instruction.
# After Tile scheduling, move excess waits onto single-wait NoOp carriers
# inserted immediately before the over-limit instruction (same engine, same
# block, so per-engine program order and wait semantics are preserved).
# ---------------------------------------------------------------------------
def _split_multiwait(nc, limit=1):
    n_new = 0
    for f in nc.m.functions:
        for blk in f.blocks:
            insts = blk.instructions
            out = []
            for ins in insts:
                si = getattr(ins, "sync_info", None)
                waits = list(si.on_wait) if si is not None else []
                if len(waits) > limit:
                    for i, w in enumerate(waits[:-limit]):
                        nop = mybir.InstNoOp(
                            name=f"{ins.name}_w{i}",
                            sync_info=mybir.SyncInfo(on_wait=[w], on_update=[]),
                            bass_nofuse=True,
                            engine=ins.engine,
                        )
                        out.append(nop)
                        n_new += 1
                    ins.sync_info = mybir.SyncInfo(
                        on_wait=waits[-limit:], on_update=list(si.on_update)
                    )
                out.append(ins)
            if len(out) != len(insts):
                blk.instructions = out
    return n_new


def _emit(tc, nc, io):
    """Emit the per-core program. io: dict of DRAM APs."""
    from contextlib import ExitStack

    x_d = io["x"]
    out_d = io["out"]

    s_outer = ExitStack()

    constp = s_outer.enter_context(tc.tile_pool(name="constp", bufs=1))
    dramp = s_outer.enter_context(tc.tile_pool(name="dramp", bufs=1, space="DRAM"))

    # ---- phase-A bulk loads, one descriptor-dense DMA each (SP queue) ----
    persp = s_outer.enter_context(tc.tile_pool(name="persp", bufs=1))
    x_all = persp.tile([128, NT, H], F32, name="x_all")
    nc.sync.dma_start(x_all[:], x_d.rearrange("(t p) h -> p t h", p=128))

    sA = ExitStack()
    wpool = sA.enter_context(tc.tile_pool(name="wpool", bufs=3))
    wk_sb = wpool.tile([128, 8, H], BF16, tag="w3", name="wk_sb")
    nc.sync.dma_start(wk_sb[:], io["wk"].rearrange("(f p) h -> p f h", p=128))

    # constants / biases
    ident = constp.tile([128, 128], BF16)
    nc.sync.dma_start(ident[:], io["ident"][:])
    ones_row = constp.tile([1, 128], BF16)
    nc.sync.dma_start(ones_row[:], io["ones_row"][:])
    # bqki: cols 0:8 = bq, 8:16 = bk, 16:48 = bi (all [128, n])
    bqki = constp.tile([128, 48], F32)
    nc.sync.dma_start(bqki[:], io["bqki"][:])
    # bvpo: [1, 3H] bf16: bv | bproj | bo
    bvpo = constp.tile([1, 3 * H], BF16)
    nc.sync.dma_start(bvpo[:], io["bvpo"][:])
    eps_t = constp.tile([128, 1], F32)
    nc.gpsimd.memset(eps_t[:], float(EPS))
    ln8_t = constp.tile([128, 1], F32)
    nc.gpsimd.memset(ln8_t[:], float(LN8))

    wq_sb = wpool.tile([128, 8, H], BF16, tag="w3", name="wq_sb")
    nc.sync.dma_start(wq_sb[:], io["wq"].rearrange("(f p) h -> p f h", p=128))
    wv_sb = wpool.tile([128, 8, H], BF16, tag="w3", name="wv_sb")
    nc.sync.dma_start(wv_sb[:], io["wv"].rearrange("(f p) h -> p f h", p=128))

    # persistent activations
    x2_all = persp.tile([128, NT, H], F32, name="x2_all")
    qT_all = persp.tile([128, 8, T], F8, name="qT_all")
    ctxT_all = persp.tile([128, 8 * T], BF16, name="ctxT_all")
    h2T_all = persp.tile([128, 8, T], BF16, name="h2T_all")
    wproj_sb = persp.tile([128, 8, H], BF16, name="wproj_sb")

    # fp8 collective buffers: K and V, each split by head-half
    cc_k_in_a = dramp.tile([512, 512], F8)
    cc_k_in_b = dramp.tile([512, 512], F8)
    cc_k_out_a = dramp.tile([G * 512, 512], F8)
    cc_k_out_b = dramp.tile([G * 512, 512], F8)
    cc_v_in_a = dramp.tile([512, 512], F8)
    cc_v_in_b = dramp.tile([512, 512], F8)
    cc_v_out_a = dramp.tile([G * 512, 512], F8)
    cc_v_out_b = dramp.tile([G * 512, 512], F8)

    def layer_norm_stats(pool, x_slice):
        """x_slice [128,H] f32 -> (rs [128,1], nmr [128,1]) in SBUF."""
        stats = pool.tile([128, 2, 6], F32, tag="ln_st")
        mv = pool.tile([128, 2], F32, tag="ln_mv")
        std = pool.tile([128, 1], F32, tag="ln_std")
        rs = pool.tile([128, 1], F32, tag="ln_rs")
        nmr = pool.tile([128, 1], F32, tag="ln_nmr")
        xc = x_slice.rearrange("p (n c) -> p n c", c=512)
        nc.vector.bn_stats(out=stats[:, 0, :], in_=xc[:, 0, :])
        nc.vector.bn_stats(out=stats[:, 1, :], in_=xc[:, 1, :])
        nc.vector.bn_aggr(out=mv[:], in_=stats[:])
        nc.scalar.activation(std[:], mv[:, 1:2], AF.Sqrt, bias=eps_t[:])
        nc.vector.reciprocal(rs[:], std[:])
        nc.vector.tensor_mul(nmr[:], mv[:, 0:1], rs[:])
        nc.vector.tensor_scalar_mul(nmr[:], nmr[:], -1.0)
        return rs, nmr

    # =====================================================================
    # Phase A: load x, LN1 -> h1, h1T (xbar transpose), kT, v, qT.
    # K/V AllGathers (by head-half) dispatched as soon as inputs land.
    # =====================================================================
    lnp = sA.enter_context(tc.tile_pool(name="lnp", bufs=2))
    h1p = sA.enter_context(tc.tile_pool(name="h1p", bufs=2))
    h1Tp = sA.enter_context(tc.tile_pool(name="h1Tp", bufs=1))
    ktp = sA.enter_context(tc.tile_pool(name="ktp", bufs=1))
    vlp = sA.enter_context(tc.tile_pool(name="vlp", bufs=2))
    mmpsA = sA.enter_context(tc.tile_pool(name="mmpsA", bufs=3, space="PSUM"))
    tpsA = None
    stgA = None
    if not USE_DMA_TRANSPOSE:
        tpsA = sA.enter_context(tc.tile_pool(name="tpsA", bufs=2, space="PSUM"))
        stgA = sA.enter_context(tc.tile_pool(name="stgA", bufs=2))

    h1T_all = h1Tp.tile([128, 8, T], BF16, name="h1T_all")
    ktA_sb = ktp.tile([128, 4, T], F8, name="ktA_sb")
    ktB_sb = ktp.tile([128, 4, T], F8, name="ktB_sb")

    def transpose_tile(dstT_all, h_tile, t):
        """h_tile [128, H] bf16 -> dstT_all[:, fb, t*128:(t+1)*128] for all fb."""
        dst = dstT_all.rearrange("p f (tt c) -> p f tt c", c=128)[:, :, t, :]
        if USE_DMA_TRANSPOSE:
            nc.scalar.dma_start_transpose(dst, h_tile)
        else:
            for fb in range(8):
                ps = tpsA.tile([128, 128], BF16, tag="tp")
                nc.tensor.transpose(ps[:], h_tile[:, fb * 128:(fb + 1) * 128], ident[:])
                nc.vector.tensor_copy(dst[:, fb, :], ps[:])

    for t in range(NT):
        rs, nmr = layer_norm_stats(lnp, x_all[:, t, :])
        h1 = h1p.tile([128, H], BF16, tag="h1")
        nc.scalar.activation(h1[:], x_all[:, t, :], AF.Identity, bias=nmr[:], scale=rs[:])
        transpose_tile(h1T_all, h1[:], t)

    def ag(cc_in, cc_out):
        nc.gpsimd.collective_compute(
            "AllGather", mybir.AluOpType.bypass, replica_groups=GROUPS,
            ins=[cc_in.opt()], outs=[cc_out.opt()])

    # kT feature-major: [128 feats(head pair), T local keys] per ct
    def emit_k_quarter(cts, dst):
        for ct in cts:
            ps = mmpsA.tile([128, T], F32, tag="mmA")
            for fb in range(8):
                nc.tensor.matmul(
                    ps[:],
                    wk_sb[:, fb, ct * 128:(ct + 1) * 128],
                    h1T_all[:, fb, :],
                    start=(fb == 0), stop=(fb == 7),
                )
            nc.vector.tensor_scalar_add(dst[:, ct % 4, :], ps[:], bqki[:, 8 + ct:9 + ct])

    # v token-major, feature half cc: [128 tok, 512]
    def emit_v_half(cc, vloc, cc_v_in):
        for t in range(NT):
            ps = mmpsA.tile([128, 512], F32, tag="mmA")
            for fb in range(8):
                nc.tensor.matmul(
                    ps[:],
                    h1T_all[:, fb, t * 128:(t + 1) * 128],
                    wv_sb[:, fb, cc * 512:(cc + 1) * 512],
                    start=(fb == 0), stop=False,
                )
            nc.tensor.matmul(ps[:], ones_row[:], bvpo[:, cc * 512:(cc + 1) * 512],
                             start=False, stop=True)
            nc.scalar.copy(vloc[:, t, :], ps[:])
        nc.sync.dma_start(
            cc_v_in.rearrange("(t p) f -> p t f", p=128), vloc[:])

    def emit_q_quarter(cts):
        for ct in cts:
            ps = mmpsA.tile([128, T], F32, tag="mmA")
            for fb in range(8):
                nc.tensor.matmul(
                    ps[:],
                    wq_sb[:, fb, ct * 128:(ct + 1) * 128],
                    h1T_all[:, fb, :],
                    start=(fb == 0), stop=(fb == 7),
                )
            nc.vector.tensor_scalar_add(qT_all[:, ct, :], ps[:], bqki[:, ct:ct + 1])

    # interleave projections with the four fp8 gathers: K-A first (hp0
    # scores), then V-A (hp0 ctx), K-B, V-B. CC runs them back-to-back.
    emit_k_quarter(range(0, 4), ktA_sb)
    nc.sync.dma_start(
        cc_k_in_a.rearrange("(c p) k -> p c k", p=128), ktA_sb[:])
    ag(cc_k_in_a, cc_k_out_a)
    vloc_a = vlp.tile([128, NT, 512], F8, tag="vloc", name="vloc_a")
    emit_v_half(0, vloc_a, cc_v_in_a)
    ag(cc_v_in_a, cc_v_out_a)
    emit_k_quarter(range(4, 8), ktB_sb)
    nc.sync.dma_start(
        cc_k_in_b.rearrange("(c p) k -> p c k", p=128), ktB_sb[:])
    ag(cc_k_in_b, cc_k_out_b)
    emit_q_quarter(range(0, 4))
    vloc_b = vlp.tile([128, NT, 512], F8, tag="vloc", name="vloc_b")
    emit_v_half(1, vloc_b, cc_v_in_b)
    ag(cc_v_in_b, cc_v_out_b)
    emit_q_quarter(range(4, 8))

    nc.sync.dma_start(wproj_sb[:], io["wproj"].rearrange("(f p) h -> p f h", p=128))

    sA.close()

    # =====================================================================
    # Phase B: attention. scores^T per key-tile (row-packed head pairs),
    # exp on ACT, ctx^T via V'=[V|ones] (M=65), fast normalize at hp end.
    # =====================================================================
    sB = ExitStack()
    ktpool = sB.enter_context(tc.tile_pool(name="ktpool", bufs=5))
    vstg = sB.enter_context(tc.tile_pool(name="vstg", bufs=3))
    spool = sB.enter_context(tc.tile_pool(name="spool", bufs=2, space="PSUM"))
    cpool = sB.enter_context(tc.tile_pool(name="cpool", bufs=2, space="PSUM"))
    ppool = sB.enter_context(tc.tile_pool(name="ppool", bufs=8))
    rpool = sB.enter_context(tc.tile_pool(name="rpool", bufs=3))
    vsbp = sB.enter_context(tc.tile_pool(name="vsbp", bufs=1))
    vsb_a = vsbp.tile([128, NKT, 8, VW], F8, name="vsb_a")
    vsb_b = vsbp.tile([128, NKT, 8, VW], F8, name="vsb_b")

    # ones columns for all key tiles (DVE; no data deps)
    nc.vector.memset(vsb_a[:, :, :, DH:VW], 1.0)
    nc.vector.memset(vsb_b[:, :, :, DH:VW], 1.0)

    def load_kt(hp):
        """kT for head pair hp: [128 feats, 4 rank-blocks, 512 keys]."""
        kt = ktpool.tile([128, G, T], F8, tag="kt", name=f"kt{hp}")
        cko, hpo = (cc_k_out_a, hp) if hp < 4 else (cc_k_out_b, hp - 4)
        src = cko.rearrange("(g c p) k -> c p g k", g=G, c=4, p=128)[hpo]
        nc.sync.dma_start(kt[:], src)
        return kt

    def load_v_half(vsb, cc_v_out):
        """Interleave gathered V [tokens, 512] into vsb [128, kt, head, VW]."""
        for g in range(G):
            vplain = vstg.tile([128, NT, 512], F8, tag="vplain", name=f"vp{g}")
            src = cc_v_out.rearrange("(g t p) f -> g p t f", g=G, p=128)[g]
            nc.sync.dma_start(vplain[:], src)
            for t in range(NT):
                kt = g * 4 + t
                nc.vector.tensor_copy(
                    vsb[:, kt, :, 0:DH],
                    vplain[:, t, :].rearrange("p (h d) -> p h d", d=DH),
                )

    def attend_hp(hp, kt_hp, vsb, pending_tail):
        """Returns a closure finishing this hp's softmax-normalize; the PE
        part of the previous hp's tail is emitted mid-loop (at kt==3) so
        the PE queue never stalls waiting on the DVE reciprocal chain."""
        hh = (hp % 4) * 2  # head-within-half base index
        cps0 = cpool.tile([128, T], F32, tag="ctx0")
        cps1 = cpool.tile([128, T], F32, tag="ctx1")

        def emit_ctx(kt, pb, first, last):
            for h, cps in enumerate((cps0, cps1)):
                nc.tensor.matmul(
                    cps[0:VW, :],
                    vsb[:, kt, hh + h, :],
                    pb[:, h * T:(h + 1) * T],
                    start=first, stop=last,
                )

        # software-pipelined: scores(kt) | ctx(kt-1) | exp(kt)
        prev = None
        for kt in range(NKT):
            g, j = kt // 4, kt % 4
            if kt == 8 and pending_tail is not None:
                pending_tail()
                pending_tail = None
            ps = spool.tile([128, 1024], F32, tag="ps", name="ps")
            nc.tensor.matmul(
                ps[:, 0:T],
                kt_hp[0:64, g, j * 128:(j + 1) * 128],
                qT_all[0:64, hp, :],
                start=True, stop=True, tile_position=(0, 0),
            )
            nc.tensor.matmul(
                ps[:, T:1024],
                kt_hp[64:128, g, j * 128:(j + 1) * 128],
                qT_all[64:128, hp, :],
                start=True, stop=True, tile_position=(64, 0),
            )
            if prev is not None:
                emit_ctx(prev[0], prev[1], prev[2], False)
            pb = ppool.tile([128, 1024], F8, tag="pb", name="pb")
            nc.scalar.activation(pb[:], ps[:], AF.Exp, bias=ln8_t[:])
            prev = (kt, pb, kt == 0)
        emit_ctx(prev[0], prev[1], prev[2], True)

        # normalize: both heads' 1/sumexp in ONE fused DVE reciprocal (the
        # [*,512] op costs the same at 1 or 2 partitions); broadcast via a
        # K=1 matmul into the same tile's spare rows 64:128 (no extra PSUM
        # bank) deferred deep into the next hp's score stream.
        se = rpool.tile([33, T], F32, tag="se")
        rc = rpool.tile([33, T], F32, tag="rc")
        rcb0 = rpool.tile([1, T], BF16, tag="rcb0")
        rcb1 = rpool.tile([1, T], BF16, tag="rcb1")
        nc.vector.tensor_copy(se[0:1, :], cps0[DH:VW, :])
        nc.vector.tensor_copy(se[32:33, :], cps1[DH:VW, :])
        nc.vector.reciprocal(rc[:], se[:])
        nc.vector.tensor_copy(rcb0[:], rc[0:1, :])
        nc.vector.tensor_copy(rcb1[:], rc[32:33, :])
        rcbs = (rcb0, rcb1)

        def tail():
            for h, cps in enumerate((cps0, cps1)):
                rbs = rpool.tile([64, T], F32, tag=f"rbs{h}")
                nc.tensor.matmul(cps[64:128, :], ones_row[:, 0:64], rcbs[h][:],
                                 start=True, stop=True, tile_position=(0, 64))
                nc.vector.tensor_copy(rbs[:], cps[64:128, :])
                nc.vector.tensor_mul(
                    ctxT_all[h * 64:(h + 1) * 64, hp * T:(hp + 1) * T],
                    cps[0:DH, :], rbs[:])
        return tail

    # half A
    kt_tiles = {}
    kt_tiles[0] = load_kt(0)
    kt_tiles[1] = load_kt(1)
    load_v_half(vsb_a, cc_v_out_a)
    kt_tiles[2] = load_kt(2)
    kt_tiles[3] = load_kt(3)
    tail = None
    for hp in range(4):
        tail = attend_hp(hp, kt_tiles[hp], vsb_a, tail)
    # half B
    kt_tiles[4] = load_kt(4)
    load_v_half(vsb_b, cc_v_out_b)
    for hp in range(4, NHP):
        if hp + 1 < NHP:
            kt_tiles[hp + 1] = load_kt(hp + 1)
        tail = attend_hp(hp, kt_tiles[hp], vsb_b, tail)
    tail()  # last hp's normalize, right before proj consumes ctxT

    sB.close()

    # =====================================================================
    # Phase C: proj (token-major) + residual -> x2, LN2 -> h2T
    # =====================================================================
    sC = ExitStack()
    lnp2 = sC.enter_context(tc.tile_pool(name="lnp2", bufs=2))
    h2p = sC.enter_context(tc.tile_pool(name="h2p", bufs=2))
    mmpsC = sC.enter_context(tc.tile_pool(name="mmpsC", bufs=3, space="PSUM"))
    tpsC = None
    stgC = None
    if not USE_DMA_TRANSPOSE:
        tpsC = sC.enter_context(tc.tile_pool(name="tpsC", bufs=2, space="PSUM"))
        stgC = sC.enter_context(tc.tile_pool(name="stgC", bufs=2))

    def transpose_tile_C(h_tile, t):
        dst = h2T_all.rearrange("p f (tt c) -> p f tt c", c=128)[:, :, t, :]
        if USE_DMA_TRANSPOSE:
            nc.scalar.dma_start_transpose(dst, h_tile)
        else:
            for fb in range(8):
                pst = tpsC.tile([128, 128], BF16, tag="tp")
                nc.tensor.transpose(pst[:], h_tile[:, fb * 128:(fb + 1) * 128], ident[:])
                nc.vector.tensor_copy(dst[:, fb, :], pst[:])

    h2_prev = None
    for t in range(NT):
        for cc in range(2):
            ps = mmpsC.tile([128, 512], F32, tag="pj")
            for hp in range(8):
                nc.tensor.matmul(
                    ps[:],
                    ctxT_all[:, hp * T + t * 128: hp * T + (t + 1) * 128],
                    wproj_sb[:, hp, cc * 512:(cc + 1) * 512],
                    start=(hp == 0), stop=False,
                )
            nc.tensor.matmul(ps[:], ones_row[:], bvpo[:, H + cc * 512:H + (cc + 1) * 512],
                             start=False, stop=True)
            nc.vector.tensor_add(
                x2_all[:, t, cc * 512:(cc + 1) * 512],
                ps[:], x_all[:, t, cc * 512:(cc + 1) * 512])
        if h2_prev is not None:
            transpose_tile_C(h2_prev[0][:], h2_prev[1])
        rs, nmr = layer_norm_stats(lnp2, x2_all[:, t, :])
        h2 = h2p.tile([128, H], BF16, tag="h2")
        nc.scalar.activation(h2[:], x2_all[:, t, :], AF.Identity, bias=nmr[:], scale=rs[:])
        h2_prev = (h2, t)
    transpose_tile_C(h2_prev[0][:], h2_prev[1])

    sC.close()

    # =====================================================================
    # Phase D+E fused: per g: wi matmuls + gelu -> h3T[g], then wo matmuls
    # for output columns 0:512 accumulate into 4 persistent psums.
    # Second pass re-reads h3T for output columns 512:1024.
    # Weight streams ride the gpsimd SWDGE queue (batched loads).
    # =====================================================================
    sD = ExitStack()
    h3p = sD.enter_context(tc.tile_pool(name="h3p", bufs=1))
    wip = sD.enter_context(tc.tile_pool(name="wip", bufs=3))
    wop = sD.enter_context(tc.tile_pool(name="wop", bufs=2))
    mmpsD = sD.enter_context(tc.tile_pool(name="mmpsD", bufs=3, space="PSUM"))
    wops = sD.enter_context(tc.tile_pool(name="wops", bufs=1, space="PSUM"))
    outp = sD.enter_context(tc.tile_pool(name="outp", bufs=2))

    h3T_all = h3p.tile([128, NG, T], BF16, name="h3T_all")

    WIB = 4   # wi groups per DMA
    WOB = 8   # wo groups per DMA

    wi_tiles = [None] * (NG // WIB)
    wo_tiles = {}

    def load_wi(b):
        wi = wip.tile([128, WIB, 8, 128], BF16, tag="wi", name=f"wi{b}")
        src = io["wi"][b * WIB:(b + 1) * WIB].rearrange("g p f c -> p g f c")
        nc.gpsimd.dma_start(wi[:], src)
        return wi

    def load_wo(cc, b):
        wo = wop.tile([128, WOB, 512], BF16, tag="wo", name=f"wo{cc}_{b}")
        src = io["wo"][:, cc * 512:(cc + 1) * 512].rearrange(
            "(bb g p) f -> bb p g f", g=WOB, p=128)[b]
        nc.gpsimd.dma_start(wo[:], src)
        return wo

    # prefetch schedule on the gpsimd queue (wi bufs=3, wo bufs=2 gate it)
    wi_tiles[0] = load_wi(0)
    wi_tiles[1] = load_wi(1)
    wo_tiles[(0, 0)] = load_wo(0, 0)
    wi_tiles[2] = load_wi(2)
    wo_tiles[(0, 1)] = load_wo(0, 1)

    psE = [wops.tile([128, 512], F32, tag=f"wo_ps{t}", name=f"wo_ps{t}") for t in range(NT)]
    for g in range(NG):
        b, gi = g // WIB, g % WIB
        if b + 3 < len(wi_tiles) and wi_tiles[b + 3] is None and gi == 0:
            wi_tiles[b + 3] = load_wi(b + 3)
        wob = g // WOB
        if gi == 1 and b % 2 == 1 and (0, b // 2 + 2) not in wo_tiles and b // 2 + 2 < 4:
            wo_tiles[(0, b // 2 + 2)] = load_wo(0, b // 2 + 2)
        wi = wi_tiles[b]
        ps = mmpsD.tile([128, T], F32, tag="wi_ps", name="wi_ps")
        for fb in range(8):
            nc.tensor.matmul(
                ps[:], wi[:, gi, fb, :], h2T_all[:, fb, :],
                start=(fb == 0), stop=(fb == 7),
            )
        nc.scalar.activation(h3T_all[:, g, :], ps[:],
                             AF.Gelu_apprx_tanh, bias=bqki[:, 16 + g:17 + g])
        wo = wo_tiles[(0, wob)]
        for t in range(NT):
            nc.tensor.matmul(
                psE[t][:],
                h3T_all[:, g, t * 128:(t + 1) * 128],
                wo[:, g % WOB, :],
                start=(g == 0), stop=False,
            )
    ostage = outp.tile([128, NT, 512], F32, tag="ostage", name="ostage0")
    for t in range(NT):
        nc.tensor.matmul(psE[t][:], ones_row[:], bvpo[:, 2 * H:2 * H + 512],
                         start=False, stop=True)
        nc.vector.tensor_add(ostage[:, t, :], psE[t][:], x2_all[:, t, 0:512])
    nc.sync.dma_start(
        out_d[:, 0:512].rearrange("(t p) f -> p t f", p=128), ostage[:])

    # second pass: output columns 512:1024
    wo_tiles[(1, 0)] = load_wo(1, 0)
    wo_tiles[(1, 1)] = load_wo(1, 1)
    psE2 = [wops.tile([128, 512], F32, tag=f"wo_ps{t}", name=f"wo2_ps{t}") for t in range(NT)]
    for g in range(NG):
        wob = g // WOB
        if g % WOB == 1 and (1, wob + 2) not in wo_tiles and wob + 2 < 4:
            wo_tiles[(1, wob + 2)] = load_wo(1, wob + 2)
        wo = wo_tiles[(1, wob)]
        for t in range(NT):
            nc.tensor.matmul(
                psE2[t][:],
                h3T_all[:, g, t * 128:(t + 1) * 128],
                wo[:, g % WOB, :],
                start=(g == 0), stop=False,
            )
    ostage2 = outp.tile([128, NT, 512], F32, tag="ostage", name="ostage1")
    for t in range(NT):
        nc.tensor.matmul(psE2[t][:], ones_row[:], bvpo[:, 2 * H + 512:3 * H],
                         start=False, stop=True)
        nc.vector.tensor_add(ostage2[:, t, :], psE2[t][:], x2_all[:, t, 512:1024])
    nc.sync.dma_start(
        out_d[:, 512:1024].rearrange("(t p) f -> p t f", p=128), ostage2[:])

    sD.close()
    s_outer.close()


def _build_program():
    nc = bass.Bass("TRN2", target_bir_lowering=False, debug=False, num_devices=NC)
    io = {}
    io["x"] = nc.dram_tensor("x", [T, H], F32, kind="ExternalInput").ap()
    io["wq"] = nc.dram_tensor("wq", [H, H], BF16, kind="ExternalInput").ap()
    io["wk"] = nc.dram_tensor("wk", [H, H], BF16, kind="ExternalInput").ap()
    io["wv"] = nc.dram_tensor("wv", [H, H], BF16, kind="ExternalInput").ap()
    io["wproj"] = nc.dram_tensor("wproj", [H, H], BF16, kind="ExternalInput").ap()
    io["wi"] = nc.dram_tensor("wi", [NG, 128, 8, 128], BF16, kind="ExternalInput").ap()
    io["wo"] = nc.dram_tensor("wo", [FFN, H], BF16, kind="ExternalInput").ap()
    io["bqki"] = nc.dram_tensor("bqki", [128, 48], F32, kind="ExternalInput").ap()
    io["bvpo"] = nc.dram_tensor("bvpo", [1, 3 * H], BF16, kind="ExternalInput").ap()
    io["ident"] = nc.dram_tensor("ident", [128, 128], BF16, kind="ExternalInput").ap()
    io["ones_row"] = nc.dram_tensor("ones_row", [1, 128], BF16, kind="ExternalInput").ap()
    io["out"] = nc.dram_tensor("out", [T, H], F32, kind="ExternalOutput").ap()
    with tile.TileContext(nc) as tc:
        _emit(tc, nc, io)
    _split_multiwait(nc)
    return nc


_PROGRAM = None
LAST_RESULTS = None


def kernel(x, ln1_scale, ln1_bias, qkv_w, qkv_b, proj_w, proj_b,
           ln2_scale, ln2_bias, wi_w, wi_b, wo_w, wo_b):
    global _PROGRAM, LAST_RESULTS
    x = np.asarray(x, np.float32)
    ln1_scale = np.asarray(ln1_scale, np.float32); ln1_bias = np.asarray(ln1_bias, np.float32)
    qkv_w = np.asarray(qkv_w, np.float32); qkv_b = np.asarray(qkv_b, np.float32)
    proj_w = np.asarray(proj_w, np.float32); proj_b = np.asarray(proj_b, np.float32)
    ln2_scale = np.asarray(ln2_scale, np.float32); ln2_bias = np.asarray(ln2_bias, np.float32)
    wi_w = np.asarray(wi_w, np.float32); wi_b = np.asarray(wi_b, np.float32)
    wo_w = np.asarray(wo_w, np.float32); wo_b = np.asarray(wo_b, np.float32)

    # fold LN affine params into the next matmul's weights/biases
    qkv_w_eff = ln1_scale[:, None] * qkv_w
    qkv_b_eff = qkv_b + ln1_bias @ qkv_w
    w3 = qkv_w_eff.reshape(H, NH, 3, DH)
    b3 = qkv_b_eff.reshape(NH, 3, DH)
    scale = 1.0 / np.sqrt(np.float32(DH))
    wq = (w3[:, :, 0, :] * scale).reshape(H, H)
    wk = w3[:, :, 1, :].reshape(H, H)
    wv = w3[:, :, 2, :].reshape(H, H)
    bq_v = (b3[:, 0, :] * scale).reshape(H)
    bk_v = b3[:, 1, :].reshape(H)
    bv_v = b3[:, 2, :].reshape(H)
    wi_eff = ln2_scale[:, None] * wi_w
    bi_v = wi_b + ln2_bias @ wi_w

    bqki = np.concatenate([
        bq_v.reshape(8, 128).T, bk_v.reshape(8, 128).T,
        bi_v.reshape(32, 128).T], axis=1).astype(np.float32)
    bvpo = np.concatenate([bv_v, proj_b, wo_b]).reshape(1, 3 * H)

    common = {
        "wq": wq.astype(NPBF16), "wk": wk.astype(NPBF16), "wv": wv.astype(NPBF16),
        "wproj": proj_w.astype(NPBF16),
        "wi": np.ascontiguousarray(
            wi_eff.astype(NPBF16).reshape(8, 128, 32, 128).transpose(2, 1, 0, 3)),
        "wo": wo_w.astype(NPBF16),
        "bqki": np.ascontiguousarray(bqki),
        "bvpo": bvpo.astype(NPBF16),
        "ident": np.eye(128, dtype=NPBF16),
        "ones_row": np.ones((1, 128), NPBF16),
    }
    x_flat = x.reshape(B * S, H)
    in_maps = []
    for c in range(NC):
        m = dict(common)
        m["x"] = np.ascontiguousarray(x_flat[c * T:(c + 1) * T, :])
        in_maps.append(m)

    if _PROGRAM is None:
        _PROGRAM = _build_program()
    r = run_bass_kernel_spmd(_PROGRAM, in_maps, list(range(NC)))
    LAST_RESULTS = r
    out = np.concatenate([r.results[c]["out"] for c in range(NC)], axis=0)
    return out.reshape(B, S, H).astype(np.float32)


# revision 21
# speedup vs baseline: 1.5350x; 1.0982x over previous
"""Trainium2 Bass kernel for a pre-LN transformer block (B=2,S=2048,H=1024,NH=16,FFN=4096).

Sharding: 8 cores, 512 tokens/core (4 cores per batch element). K/V are
exchanged within each batch group via four 4-rank AllGathers (K and V each in
two head-halves) so attention on early head-pairs overlaps the later gathers.
All matmuls run in bf16 on the PE array with fp32 PSUM accumulation; LayerNorm
statistics, residuals and the final output stay fp32.

Self-contained: hardcodes shapes; builds the Bass program once and runs it via
run_bass_kernel_spmd on cores 0-7.
"""

import sys

for _p in ("/root/.axon_site/_ro/trn_rl_repo", "/opt/trn_rl_repo"):
    if _p not in sys.path:
        sys.path.append(_p)

import numpy as np
import ml_dtypes

# If BASS_TRACE is set but the axon NTFF hook module is missing, the trace
# path would crash on import; pre-register a no-op hook shim so tracing
# degrades gracefully instead.
try:
    import antenv.axon_hooks  # noqa: F401
except ImportError:
    import types as _types
    _m = _types.ModuleType("antenv.axon_hooks")
    _m._hook = None
    _m.get_axon_ntff_profile_hook = lambda: _m._hook
    _m.set_axon_ntff_profile_hook = lambda h: setattr(_m, "_hook", h)
    sys.modules["antenv.axon_hooks"] = _m

import bass_rust
import concourse.bass as bass
import concourse.mybir as mybir
import concourse.tile as tile
from concourse.bass_utils import run_bass_kernel_spmd

BF16 = mybir.dt.bfloat16
F32 = mybir.dt.float32
F8 = mybir.dt.float8e4
LN8 = 2.0794415416798357  # exp scale: probs stored as 8*exp(s), cancels in normalize
AF = mybir.ActivationFunctionType
NPBF16 = np.dtype(ml_dtypes.bfloat16)
NPF8 = np.dtype(ml_dtypes.float8_e4m3fn)
DR = mybir.MatmulPerfMode.DoubleRow

B, S, H, NH, DH, FFN = 2, 2048, 1024, 16, 64, 4096
NC = 8                      # cores
T = 512                     # tokens per core
NT = T // 128               # token tiles per core (4)
GROUPS = [[0, 1, 2, 3], [4, 5, 6, 7]]
G = 4                       # cores per batch group
NKT = 16                    # key tiles per batch (4 ranks x 4)
NHP = NH // 2               # head pairs (8)
EPS = 1e-3
VW = DH + 2                 # 66: V cols + 2 ones cols (even for dual-fp8 ldweights)
NG = FFN // 128             # 32 ffn row-groups

USE_DMA_TRANSPOSE = False   # xbar DMA transpose serializes ~6us/tile; PE wins

# ---------------------------------------------------------------------------
# Workaround: this walrus build rejects >1 inline sync-wait per instruction.
# After Tile scheduling, move excess waits onto single-wait NoOp carriers
# inserted immediately before the over-limit instruction (same engine, same
# block, so per-engine program order and wait semantics are preserved).
# ---------------------------------------------------------------------------
def _split_multiwait(nc, limit=1):
    n_new = 0
    for f in nc.m.functions:
        for blk in f.blocks:
            insts = blk.instructions
            out = []
            for ins in insts:
                si = getattr(ins, "sync_info", None)
                waits = list(si.on_wait) if si is not None else []
                if len(waits) > limit:
                    for i, w in enumerate(waits[:-limit]):
                        nop = mybir.InstNoOp(
                            name=f"{ins.name}_w{i}",
                            sync_info=mybir.SyncInfo(on_wait=[w], on_update=[]),
                            bass_nofuse=True,
                            engine=ins.engine,
                        )
                        out.append(nop)
                        n_new += 1
                    ins.sync_info = mybir.SyncInfo(
                        on_wait=waits[-limit:], on_update=list(si.on_update)
                    )
                out.append(ins)
            if len(out) != len(insts):
                blk.instructions = out
    return n_new


def _emit(tc, nc, io):
    """Emit the per-core program. io: dict of DRAM APs."""
    from contextlib import ExitStack

    x_d = io["x"]
    out_d = io["out"]

    s_outer = ExitStack()

    constp = s_outer.enter_context(tc.tile_pool(name="constp", bufs=1))
    dramp = s_outer.enter_context(tc.tile_pool(name="dramp", bufs=1, space="DRAM"))

    # ---- phase-A bulk loads, one descriptor-dense DMA each (SP queue) ----
    persp = s_outer.enter_context(tc.tile_pool(name="persp", bufs=1))
    x_all = persp.tile([128, NT, H], F32, name="x_all")
    nc.sync.dma_start(x_all[:], x_d.rearrange("(t p) h -> p t h", p=128))

    sA = ExitStack()
    wpool = sA.enter_context(tc.tile_pool(name="wpool", bufs=3))
    wk_sb = wpool.tile([128, 8, H], F8, tag="w3", name="wk_sb")
    nc.sync.dma_start(wk_sb[:], io["wk"].rearrange("(f p) h -> p f h", p=128))

    # constants / biases
    ident = constp.tile([128, 128], BF16)
    nc.sync.dma_start(ident[:], io["ident"][:])
    ident_f8 = constp.tile([128, 128], F8)
    nc.sync.dma_start(ident_f8[:], io["ident_f8"][:])
    ones_row = constp.tile([1, 128], BF16)
    nc.sync.dma_start(ones_row[:], io["ones_row"][:])
    # bqki: cols 0:8 = bq, 8:16 = bk, 16:48 = bi (all [128, n])
    bqki = constp.tile([128, 48], F32)
    nc.sync.dma_start(bqki[:], io["bqki"][:])
    # bvpo: [1, 3H] bf16: bv | bproj | bo
    bvpo = constp.tile([1, 3 * H], BF16)
    nc.sync.dma_start(bvpo[:], io["bvpo"][:])
    eps_t = constp.tile([128, 1], F32)
    nc.gpsimd.memset(eps_t[:], float(EPS))
    ln8_t = constp.tile([128, 1], F32)
    nc.gpsimd.memset(ln8_t[:], float(LN8))

    wq_sb = wpool.tile([128, 8, H], F8, tag="w3", name="wq_sb")
    nc.sync.dma_start(wq_sb[:], io["wq"].rearrange("(f p) h -> p f h", p=128))
    wv_sb = wpool.tile([128, 8, H], F8, tag="w3", name="wv_sb")
    nc.sync.dma_start(wv_sb[:], io["wv"].rearrange("(f p) h -> p f h", p=128))

    # persistent activations
    x2_all = persp.tile([128, NT, H], F32, name="x2_all")
    qT_all = persp.tile([128, 8, T], F8, name="qT_all")
    ctxT_all = persp.tile([128, 8 * T], F8, name="ctxT_all")
    h2T_all = persp.tile([128, 8, T], BF16, name="h2T_all")
    wproj_sb = persp.tile([128, 8, H], F8, name="wproj_sb")

    # fp8 collective buffers: K and V, each split by head-half
    cc_k_in_a = dramp.tile([512, 512], F8)
    cc_k_in_b = dramp.tile([512, 512], F8)
    cc_k_out_a = dramp.tile([G * 512, 512], F8)
    cc_k_out_b = dramp.tile([G * 512, 512], F8)
    cc_v_in_a = dramp.tile([512, 512], F8)
    cc_v_in_b = dramp.tile([512, 512], F8)
    cc_v_out_a = dramp.tile([G * 512, 512], F8)
    cc_v_out_b = dramp.tile([G * 512, 512], F8)

    def layer_norm_stats(pool, x_slice):
        """x_slice [128,H] f32 -> (rs [128,1], nmr [128,1]) in SBUF."""
        stats = pool.tile([128, 2, 6], F32, tag="ln_st")
        mv = pool.tile([128, 2], F32, tag="ln_mv")
        std = pool.tile([128, 1], F32, tag="ln_std")
        rs = pool.tile([128, 1], F32, tag="ln_rs")
        nmr = pool.tile([128, 1], F32, tag="ln_nmr")
        xc = x_slice.rearrange("p (n c) -> p n c", c=512)
        nc.vector.bn_stats(out=stats[:, 0, :], in_=xc[:, 0, :])
        nc.vector.bn_stats(out=stats[:, 1, :], in_=xc[:, 1, :])
        nc.vector.bn_aggr(out=mv[:], in_=stats[:])
        nc.scalar.activation(std[:], mv[:, 1:2], AF.Sqrt, bias=eps_t[:])
        nc.vector.reciprocal(rs[:], std[:])
        nc.vector.tensor_mul(nmr[:], mv[:, 0:1], rs[:])
        nc.vector.tensor_scalar_mul(nmr[:], nmr[:], -1.0)
        return rs, nmr

    # =====================================================================
    # Phase A: load x, LN1 -> h1, h1T (xbar transpose), kT, v, qT.
    # K/V AllGathers (by head-half) dispatched as soon as inputs land.
    # =====================================================================
    lnp = sA.enter_context(tc.tile_pool(name="lnp", bufs=2))
    h1p = sA.enter_context(tc.tile_pool(name="h1p", bufs=2))
    h1Tp = sA.enter_context(tc.tile_pool(name="h1Tp", bufs=1))
    ktp = sA.enter_context(tc.tile_pool(name="ktp", bufs=1))
    vlp = sA.enter_context(tc.tile_pool(name="vlp", bufs=2))
    mmpsA = sA.enter_context(tc.tile_pool(name="mmpsA", bufs=3, space="PSUM"))
    tpsA = None
    stgA = None
    if not USE_DMA_TRANSPOSE:
        tpsA = sA.enter_context(tc.tile_pool(name="tpsA", bufs=2, space="PSUM"))
        stgA = sA.enter_context(tc.tile_pool(name="stgA", bufs=2))

    h1T_all = h1Tp.tile([128, 8, T], F8, name="h1T_all")
    ktA_sb = ktp.tile([128, 4, T], F8, name="ktA_sb")
    ktB_sb = ktp.tile([128, 4, T], F8, name="ktB_sb")

    def transpose_tile(dstT_all, h_tile, t):
        """h_tile [128, H] bf16 -> dstT_all[:, fb, t*128:(t+1)*128] for all fb."""
        dst = dstT_all.rearrange("p f (tt c) -> p f tt c", c=128)[:, :, t, :]
        if USE_DMA_TRANSPOSE:
            nc.scalar.dma_start_transpose(dst, h_tile)
        else:
            for fb in range(8):
                ps = tpsA.tile([128, 128], BF16, tag="tp")
                nc.tensor.transpose(ps[:], h_tile[:, fb * 128:(fb + 1) * 128], ident[:])
                nc.vector.tensor_copy(dst[:, fb, :], ps[:])

    for t in range(NT):
        rs, nmr = layer_norm_stats(lnp, x_all[:, t, :])
        h1 = h1p.tile([128, H], BF16, tag="h1")
        nc.scalar.activation(h1[:], x_all[:, t, :], AF.Identity, bias=nmr[:], scale=rs[:])
        transpose_tile(h1T_all, h1[:], t)

    def ag(cc_in, cc_out):
        nc.gpsimd.collective_compute(
            "AllGather", mybir.AluOpType.bypass, replica_groups=GROUPS,
            ins=[cc_in.opt()], outs=[cc_out.opt()])

    # kT feature-major: [128 feats(head pair), T local keys] per ct
    def emit_k_quarter(cts, dst):
        for ct in cts:
            ps = mmpsA.tile([128, T], F32, tag="mmA")
            for j in range(4):
                nc.tensor.matmul(
                    ps[:],
                    wk_sb[:, 2 * j:2 * j + 2, ct * 128:(ct + 1) * 128],
                    h1T_all[:, 2 * j:2 * j + 2, :],
                    start=(j == 0), stop=(j == 3), perf_mode=DR,
                )
            nc.vector.tensor_scalar(dst[:, ct % 4, :], ps[:], 1.0 / 16, bqki[:, 8 + ct:9 + ct],
                                    op0=mybir.AluOpType.mult, op1=mybir.AluOpType.add)

    # v token-major, feature half cc: [128 tok, 512]
    def emit_v_half(cc, vloc, cc_v_in):
        for t in range(NT):
            ps = mmpsA.tile([128, 512], F32, tag="mmA")
            for j in range(4):
                nc.tensor.matmul(
                    ps[:],
                    h1T_all[:, 2 * j:2 * j + 2, t * 128:(t + 1) * 128],
                    wv_sb[:, 2 * j:2 * j + 2, cc * 512:(cc + 1) * 512],
                    start=(j == 0), stop=False, perf_mode=DR,
                )
            nc.tensor.matmul(ps[:], ones_row[:], bvpo[:, cc * 512:(cc + 1) * 512],
                             start=False, stop=True)
            nc.scalar.activation(vloc[:, t, :], ps[:], AF.Copy, scale=1.0 / 16)
        nc.sync.dma_start(
            cc_v_in.rearrange("(t p) f -> p t f", p=128), vloc[:])

    def emit_q_quarter(cts):
        for ct in cts:
            ps = mmpsA.tile([128, T], F32, tag="mmA")
            for j in range(4):
                nc.tensor.matmul(
                    ps[:],
                    wq_sb[:, 2 * j:2 * j + 2, ct * 128:(ct + 1) * 128],
                    h1T_all[:, 2 * j:2 * j + 2, :],
                    start=(j == 0), stop=(j == 3), perf_mode=DR,
                )
            nc.vector.tensor_scalar(qT_all[:, ct, :], ps[:], 1.0 / 16, bqki[:, ct:ct + 1],
                                    op0=mybir.AluOpType.mult, op1=mybir.AluOpType.add)

    # interleave projections with the four fp8 gathers: K-A first (hp0
    # scores), then V-A (hp0 ctx), K-B, V-B. CC runs them back-to-back.
    emit_k_quarter(range(0, 4), ktA_sb)
    nc.sync.dma_start(
        cc_k_in_a.rearrange("(c p) k -> p c k", p=128), ktA_sb[:])
    ag(cc_k_in_a, cc_k_out_a)
    vloc_a = vlp.tile([128, NT, 512], F8, tag="vloc", name="vloc_a")
    emit_v_half(0, vloc_a, cc_v_in_a)
    ag(cc_v_in_a, cc_v_out_a)
    emit_k_quarter(range(4, 8), ktB_sb)
    nc.sync.dma_start(
        cc_k_in_b.rearrange("(c p) k -> p c k", p=128), ktB_sb[:])
    ag(cc_k_in_b, cc_k_out_b)
    emit_q_quarter(range(0, 4))
    vloc_b = vlp.tile([128, NT, 512], F8, tag="vloc", name="vloc_b")
    emit_v_half(1, vloc_b, cc_v_in_b)
    ag(cc_v_in_b, cc_v_out_b)
    emit_q_quarter(range(4, 8))

    nc.sync.dma_start(wproj_sb[:], io["wproj"].rearrange("(f p) h -> p f h", p=128))

    sA.close()

    # =====================================================================
    # Phase B: attention. scores^T per key-tile (row-packed head pairs),
    # exp on ACT, ctx^T via V'=[V|ones] (M=65), fast normalize at hp end.
    # =====================================================================
    sB = ExitStack()
    ktpool = sB.enter_context(tc.tile_pool(name="ktpool", bufs=5))
    vstg = sB.enter_context(tc.tile_pool(name="vstg", bufs=3))
    spool = sB.enter_context(tc.tile_pool(name="spool", bufs=2, space="PSUM"))
    cpool = sB.enter_context(tc.tile_pool(name="cpool", bufs=2, space="PSUM"))
    ppool = sB.enter_context(tc.tile_pool(name="ppool", bufs=8))
    rpool = sB.enter_context(tc.tile_pool(name="rpool", bufs=3))
    vsbp = sB.enter_context(tc.tile_pool(name="vsbp", bufs=1))
    vsb_a = vsbp.tile([128, NKT, 8, VW], F8, name="vsb_a")
    vsb_b = vsbp.tile([128, NKT, 8, VW], F8, name="vsb_b")

    # ones columns for all key tiles (DVE; no data deps)
    nc.vector.memset(vsb_a[:, :, :, DH:VW], 1.0)
    nc.vector.memset(vsb_b[:, :, :, DH:VW], 1.0)

    def load_kt(hp):
        """kT for head pair hp: [128 feats, 4 rank-blocks, 512 keys]."""
        kt = ktpool.tile([128, G, T], F8, tag="kt", name=f"kt{hp}")
        cko, hpo = (cc_k_out_a, hp) if hp < 4 else (cc_k_out_b, hp - 4)
        src = cko.rearrange("(g c p) k -> c p g k", g=G, c=4, p=128)[hpo]
        nc.sync.dma_start(kt[:], src)
        return kt

    def load_v_half(vsb, cc_v_out):
        """Interleave gathered V [tokens, 512] into vsb [128, kt, head, VW]."""
        for g in range(G):
            vplain = vstg.tile([128, NT, 512], F8, tag="vplain", name=f"vp{g}")
            src = cc_v_out.rearrange("(g t p) f -> g p t f", g=G, p=128)[g]
            nc.sync.dma_start(vplain[:], src)
            for t in range(NT):
                kt = g * 4 + t
                nc.vector.tensor_copy(
                    vsb[:, kt, :, 0:DH],
                    vplain[:, t, :].rearrange("p (h d) -> p h d", d=DH),
                )

    def attend_hp(hp, kt_hp, vsb, pending_tail):
        """Returns a closure finishing this hp's softmax-normalize; the PE
        part of the previous hp's tail is emitted mid-loop (at kt==3) so
        the PE queue never stalls waiting on the DVE reciprocal chain."""
        hh = (hp % 4) * 2  # head-within-half base index
        cps0 = cpool.tile([128, T], F32, tag="ctx0")
        cps1 = cpool.tile([128, T], F32, tag="ctx1")

        def emit_ctx(pi, pb2, first, last):
            for h, cps in enumerate((cps0, cps1)):
                nc.tensor.matmul(
                    cps[0:VW, :],
                    vsb[:, 2 * pi:2 * pi + 2, hh + h, :],
                    pb2[:, :, h, :],
                    start=first, stop=last, perf_mode=DR,
                )

        # software-pipelined over kt-PAIRS: scores+exp fill a [128,2,2,512]
        # pair tile; ctx runs as fp8 DoubleRow (two key tiles accumulated
        # per instruction), lagging 2 pairs to ride out V-gather latency.
        NPAIR = NKT // 2
        pend = []
        for pi in range(NPAIR):
            if pi == 4 and pending_tail is not None:
                pending_tail()
                pending_tail = None
            pb2 = ppool.tile([128, 2, 2, T], F8, tag="pb", name="pb")
            for j in range(2):
                kt = 2 * pi + j
                g, jj = kt // 4, kt % 4
                ps = spool.tile([128, 1024], F32, tag="ps", name="ps")
                nc.tensor.matmul(
                    ps[:, 0:T],
                    kt_hp[0:64, g, jj * 128:(jj + 1) * 128],
                    qT_all[0:64, hp, :],
                    start=True, stop=True, tile_position=(0, 0),
                )
                nc.tensor.matmul(
                    ps[:, T:1024],
                    kt_hp[64:128, g, jj * 128:(jj + 1) * 128],
                    qT_all[64:128, hp, :],
                    start=True, stop=True, tile_position=(64, 0),
                )
                if j == 1 and len(pend) >= 2:
                    emit_ctx(*pend.pop(0))
                nc.scalar.activation(pb2[:, j, :, :], ps[:], AF.Exp, bias=ln8_t[:])
            pend.append((pi, pb2, pi == 0, pi == NPAIR - 1))
        while pend:
            emit_ctx(*pend.pop(0))

        # normalize: both heads' 1/sumexp in ONE fused DVE reciprocal (the
        # [*,512] op costs the same at 1 or 2 partitions); broadcast via a
        # K=1 matmul into the same tile's spare rows 64:128 (no extra PSUM
        # bank) deferred deep into the next hp's score stream.
        se = rpool.tile([33, T], F32, tag="se")
        rc = rpool.tile([33, T], F32, tag="rc")
        rcb0 = rpool.tile([1, T], BF16, tag="rcb0")
        rcb1 = rpool.tile([1, T], BF16, tag="rcb1")
        nc.vector.tensor_copy(se[0:1, :], cps0[DH:DH + 1, :])
        nc.vector.tensor_copy(se[32:33, :], cps1[DH:DH + 1, :])
        nc.vector.reciprocal(rc[:], se[:])
        nc.vector.tensor_scalar_mul(rcb0[:], rc[0:1, :], 16.0)
        nc.vector.tensor_scalar_mul(rcb1[:], rc[32:33, :], 16.0)
        rcbs = (rcb0, rcb1)

        def tail():
            for h, cps in enumerate((cps0, cps1)):
                rbs = rpool.tile([64, T], F32, tag=f"rbs{h}")
                nc.tensor.matmul(cps[64:128, :], ones_row[:, 0:64], rcbs[h][:],
                                 start=True, stop=True, tile_position=(0, 64))
                nc.vector.tensor_copy(rbs[:], cps[64:128, :])
                nc.vector.tensor_mul(
                    ctxT_all[h * 64:(h + 1) * 64, hp * T:(hp + 1) * T],
                    cps[0:DH, :], rbs[:])
        return tail

    # half A
    kt_tiles = {}
    kt_tiles[0] = load_kt(0)
    kt_tiles[1] = load_kt(1)
    load_v_half(vsb_a, cc_v_out_a)
    kt_tiles[2] = load_kt(2)
    kt_tiles[3] = load_kt(3)
    tail = None
    for hp in range(4):
        tail = attend_hp(hp, kt_tiles[hp], vsb_a, tail)
    # half B
    kt_tiles[4] = load_kt(4)
    load_v_half(vsb_b, cc_v_out_b)
    for hp in range(4, NHP):
        if hp + 1 < NHP:
            kt_tiles[hp + 1] = load_kt(hp + 1)
        tail = attend_hp(hp, kt_tiles[hp], vsb_b, tail)
    tail()  # last hp's normalize, right before proj consumes ctxT

    sB.close()

    # =====================================================================
    # Phase C: proj (token-major) + residual -> x2, LN2 -> h2T
    # =====================================================================
    sC = ExitStack()
    lnp2 = sC.enter_context(tc.tile_pool(name="lnp2", bufs=2))
    h2p = sC.enter_context(tc.tile_pool(name="h2p", bufs=2))
    mmpsC = sC.enter_context(tc.tile_pool(name="mmpsC", bufs=3, space="PSUM"))
    tpsC = None
    stgC = None
    if not USE_DMA_TRANSPOSE:
        tpsC = sC.enter_context(tc.tile_pool(name="tpsC", bufs=2, space="PSUM"))
        stgC = sC.enter_context(tc.tile_pool(name="stgC", bufs=2))

    def transpose_tile_C(h_tile, t):
        dst = h2T_all.rearrange("p f (tt c) -> p f tt c", c=128)[:, :, t, :]
        if USE_DMA_TRANSPOSE:
            nc.scalar.dma_start_transpose(dst, h_tile)
        else:
            for fb in range(8):
                pst = tpsC.tile([128, 128], BF16, tag="tp")
                nc.tensor.transpose(pst[:], h_tile[:, fb * 128:(fb + 1) * 128], ident[:])
                nc.vector.tensor_copy(dst[:, fb, :], pst[:])

    h2_prev = None
    for t in range(NT):
        ctxTv = ctxT_all.rearrange("p (hp tt) -> p hp tt", tt=T)
        for cc in range(2):
            ps = mmpsC.tile([128, 512], F32, tag="pj")
            for j in range(4):
                nc.tensor.matmul(
                    ps[:],
                    ctxTv[:, 2 * j:2 * j + 2, t * 128:(t + 1) * 128],
                    wproj_sb[:, 2 * j:2 * j + 2, cc * 512:(cc + 1) * 512],
                    start=(j == 0), stop=False, perf_mode=DR,
                )
            nc.tensor.matmul(ps[:], ones_row[:], bvpo[:, H + cc * 512:H + (cc + 1) * 512],
                             start=False, stop=True)
            nc.vector.scalar_tensor_tensor(
                out=x2_all[:, t, cc * 512:(cc + 1) * 512],
                in0=ps[:], scalar=1.0 / 256,
                in1=x_all[:, t, cc * 512:(cc + 1) * 512],
                op0=mybir.AluOpType.mult, op1=mybir.AluOpType.add)
        if h2_prev is not None:
            transpose_tile_C(h2_prev[0][:], h2_prev[1])
        rs, nmr = layer_norm_stats(lnp2, x2_all[:, t, :])
        h2 = h2p.tile([128, H], BF16, tag="h2")
        nc.scalar.activation(h2[:], x2_all[:, t, :], AF.Identity, bias=nmr[:], scale=rs[:])
        h2_prev = (h2, t)
    transpose_tile_C(h2_prev[0][:], h2_prev[1])

    sC.close()

    # =====================================================================
    # Phase D+E fused: per g: wi matmuls + gelu -> h3T[g], then wo matmuls
    # for output columns 0:512 accumulate into 4 persistent psums.
    # Second pass re-reads h3T for output columns 512:1024.
    # Weight streams ride the gpsimd SWDGE queue (batched loads).
    # =====================================================================
    sD = ExitStack()
    h3p = sD.enter_context(tc.tile_pool(name="h3p", bufs=1))
    wip = sD.enter_context(tc.tile_pool(name="wip", bufs=3))
    wop = sD.enter_context(tc.tile_pool(name="wop", bufs=2))
    mmpsD = sD.enter_context(tc.tile_pool(name="mmpsD", bufs=3, space="PSUM"))
    wops = sD.enter_context(tc.tile_pool(name="wops", bufs=1, space="PSUM"))
    outp = sD.enter_context(tc.tile_pool(name="outp", bufs=2))

    h3T_all = h3p.tile([128, NG, T], BF16, name="h3T_all")

    WIB = 4   # wi groups per DMA
    WOB = 8   # wo groups per DMA

    wi_tiles = [None] * (NG // WIB)
    wo_tiles = {}

    def load_wi(b):
        wi = wip.tile([128, WIB, 8, 128], BF16, tag="wi", name=f"wi{b}")
        src = io["wi"][b * WIB:(b + 1) * WIB].rearrange("g p f c -> p g f c")
        nc.gpsimd.dma_start(wi[:], src)
        return wi

    def load_wo(cc, b):
        wo = wop.tile([128, WOB, 512], BF16, tag="wo", name=f"wo{cc}_{b}")
        src = io["wo"][:, cc * 512:(cc + 1) * 512].rearrange(
            "(bb g p) f -> bb p g f", g=WOB, p=128)[b]
        nc.gpsimd.dma_start(wo[:], src)
        return wo

    # prefetch schedule on the gpsimd queue (wi bufs=3, wo bufs=2 gate it)
    wi_tiles[0] = load_wi(0)
    wi_tiles[1] = load_wi(1)
    wo_tiles[(0, 0)] = load_wo(0, 0)
    wi_tiles[2] = load_wi(2)
    wo_tiles[(0, 1)] = load_wo(0, 1)

    psE = [wops.tile([128, 512], F32, tag=f"wo_ps{t}", name=f"wo_ps{t}") for t in range(NT)]
    for g in range(NG):
        b, gi = g // WIB, g % WIB
        if b + 3 < len(wi_tiles) and wi_tiles[b + 3] is None and gi == 0:
            wi_tiles[b + 3] = load_wi(b + 3)
        wob = g // WOB
        if gi == 1 and b % 2 == 1 and (0, b // 2 + 2) not in wo_tiles and b // 2 + 2 < 4:
            wo_tiles[(0, b // 2 + 2)] = load_wo(0, b // 2 + 2)
        wi = wi_tiles[b]
        ps = mmpsD.tile([128, T], F32, tag="wi_ps", name="wi_ps")
        for fb in range(8):
            nc.tensor.matmul(
                ps[:], wi[:, gi, fb, :], h2T_all[:, fb, :],
                start=(fb == 0), stop=(fb == 7),
            )
        nc.scalar.activation(h3T_all[:, g, :], ps[:],
                             AF.Gelu_apprx_tanh, bias=bqki[:, 16 + g:17 + g])
        wo = wo_tiles[(0, wob)]
        for t in range(NT):
            nc.tensor.matmul(
                psE[t][:],
                h3T_all[:, g, t * 128:(t + 1) * 128],
                wo[:, g % WOB, :],
                start=(g == 0), stop=False,
            )
    ostage = outp.tile([128, NT, 512], F32, tag="ostage", name="ostage0")
    for t in range(NT):
        nc.tensor.matmul(psE[t][:], ones_row[:], bvpo[:, 2 * H:2 * H + 512],
                         start=False, stop=True)
        nc.vector.tensor_add(ostage[:, t, :], psE[t][:], x2_all[:, t, 0:512])
    nc.sync.dma_start(
        out_d[:, 0:512].rearrange("(t p) f -> p t f", p=128), ostage[:])

    # second pass: output columns 512:1024
    wo_tiles[(1, 0)] = load_wo(1, 0)
    wo_tiles[(1, 1)] = load_wo(1, 1)
    psE2 = [wops.tile([128, 512], F32, tag=f"wo_ps{t}", name=f"wo2_ps{t}") for t in range(NT)]
    for g in range(NG):
        wob = g // WOB
        if g % WOB == 1 and (1, wob + 2) not in wo_tiles and wob + 2 < 4:
            wo_tiles[(1, wob + 2)] = load_wo(1, wob + 2)
        wo = wo_tiles[(1, wob)]
        for t in range(NT):
            nc.tensor.matmul(
                psE2[t][:],
                h3T_all[:, g, t * 128:(t + 1) * 128],
                wo[:, g % WOB, :],
                start=(g == 0), stop=False,
            )
    ostage2 = outp.tile([128, NT, 512], F32, tag="ostage", name="ostage1")
    for t in range(NT):
        nc.tensor.matmul(psE2[t][:], ones_row[:], bvpo[:, 2 * H + 512:3 * H],
                         start=False, stop=True)
        nc.vector.tensor_add(ostage2[:, t, :], psE2[t][:], x2_all[:, t, 512:1024])
    nc.sync.dma_start(
        out_d[:, 512:1024].rearrange("(t p) f -> p t f", p=128), ostage2[:])

    sD.close()
    s_outer.close()


def _build_program():
    nc = bass.Bass("TRN2", target_bir_lowering=False, debug=False, num_devices=NC)
    io = {}
    io["x"] = nc.dram_tensor("x", [T, H], F32, kind="ExternalInput").ap()
    io["wq"] = nc.dram_tensor("wq", [H, H], F8, kind="ExternalInput").ap()
    io["wk"] = nc.dram_tensor("wk", [H, H], F8, kind="ExternalInput").ap()
    io["wv"] = nc.dram_tensor("wv", [H, H], F8, kind="ExternalInput").ap()
    io["wproj"] = nc.dram_tensor("wproj", [H, H], F8, kind="ExternalInput").ap()
    io["wi"] = nc.dram_tensor("wi", [NG, 128, 8, 128], BF16, kind="ExternalInput").ap()
    io["wo"] = nc.dram_tensor("wo", [FFN, H], BF16, kind="ExternalInput").ap()
    io["bqki"] = nc.dram_tensor("bqki", [128, 48], F32, kind="ExternalInput").ap()
    io["bvpo"] = nc.dram_tensor("bvpo", [1, 3 * H], BF16, kind="ExternalInput").ap()
    io["ident"] = nc.dram_tensor("ident", [128, 128], BF16, kind="ExternalInput").ap()
    io["ident_f8"] = nc.dram_tensor("ident_f8", [128, 128], F8, kind="ExternalInput").ap()
    io["ones_row"] = nc.dram_tensor("ones_row", [1, 128], BF16, kind="ExternalInput").ap()
    io["out"] = nc.dram_tensor("out", [T, H], F32, kind="ExternalOutput").ap()
    with tile.TileContext(nc) as tc:
        _emit(tc, nc, io)
    _split_multiwait(nc)
    return nc


_PROGRAM = None
LAST_RESULTS = None


def kernel(x, ln1_scale, ln1_bias, qkv_w, qkv_b, proj_w, proj_b,
           ln2_scale, ln2_bias, wi_w, wi_b, wo_w, wo_b):
    global _PROGRAM, LAST_RESULTS
    x = np.asarray(x, np.float32)
    ln1_scale = np.asarray(ln1_scale, np.float32); ln1_bias = np.asarray(ln1_bias, np.float32)
    qkv_w = np.asarray(qkv_w, np.float32); qkv_b = np.asarray(qkv_b, np.float32)
    proj_w = np.asarray(proj_w, np.float32); proj_b = np.asarray(proj_b, np.float32)
    ln2_scale = np.asarray(ln2_scale, np.float32); ln2_bias = np.asarray(ln2_bias, np.float32)
    wi_w = np.asarray(wi_w, np.float32); wi_b = np.asarray(wi_b, np.float32)
    wo_w = np.asarray(wo_w, np.float32); wo_b = np.asarray(wo_b, np.float32)

    # fold LN affine params into the next matmul's weights/biases
    qkv_w_eff = ln1_scale[:, None] * qkv_w
    qkv_b_eff = qkv_b + ln1_bias @ qkv_w
    w3 = qkv_w_eff.reshape(H, NH, 3, DH)
    b3 = qkv_b_eff.reshape(NH, 3, DH)
    scale = 1.0 / np.sqrt(np.float32(DH))
    wq = (w3[:, :, 0, :] * scale).reshape(H, H)
    wk = w3[:, :, 1, :].reshape(H, H)
    wv = w3[:, :, 2, :].reshape(H, H)
    bq_v = (b3[:, 0, :] * scale).reshape(H)
    bk_v = b3[:, 1, :].reshape(H)
    bv_v = b3[:, 2, :].reshape(H)
    wi_eff = ln2_scale[:, None] * wi_w
    bi_v = wi_b + ln2_bias @ wi_w

    bqki = np.concatenate([
        bq_v.reshape(8, 128).T, bk_v.reshape(8, 128).T,
        bi_v.reshape(32, 128).T], axis=1).astype(np.float32)
    bvpo = np.concatenate([bv_v * 16, proj_b * 256, wo_b]).reshape(1, 3 * H)

    common = {
        "wq": (wq * 16).astype(NPF8), "wk": (wk * 16).astype(NPF8),
        "wv": (wv * 16).astype(NPF8),
        "wproj": (proj_w * 16).astype(NPF8),
        "wi": np.ascontiguousarray(
            wi_eff.astype(NPBF16).reshape(8, 128, 32, 128).transpose(2, 1, 0, 3)),
        "wo": wo_w.astype(NPBF16),
        "bqki": np.ascontiguousarray(bqki),
        "bvpo": bvpo.astype(NPBF16),
        "ident": np.eye(128, dtype=NPBF16),
        "ident_f8": np.eye(128, dtype=NPF8),
        "ones_row": np.ones((1, 128), NPBF16),
    }
    x_flat = x.reshape(B * S, H)
    in_maps = []
    for c in range(NC):
        m = dict(common)
        m["x"] = np.ascontiguousarray(x_flat[c * T:(c + 1) * T, :])
        in_maps.append(m)

    if _PROGRAM is None:
        _PROGRAM = _build_program()
    r = run_bass_kernel_spmd(_PROGRAM, in_maps, list(range(NC)))
    LAST_RESULTS = r
    out = np.concatenate([r.results[c]["out"] for c in range(NC)], axis=0)
    return out.reshape(B, S, H).astype(np.float32)
